# revision 1
# baseline (speedup 1.0000x reference)
"""Multi-head attention (AttnProcessor2_0) on 8 TRN2 NeuronCores.

Problem: B=2, S=4096, C=640, H=10, Dh=64.
  q/k/v = hs @ W{q,k,v}.T ; per-head scores = q k^T / 8 ; softmax ;
  out = probs v ; y = out @ Wo.T + b_out + hs

Sharding (no collectives): core c -> batch b=c//4, query block g=c%4
(1024 queries).  Each core recomputes full K/V for its batch (head-dim
on partitions), computes its own S/4 x S attention block, output
projection, bias+residual.  Host passes hidden states TRANSPOSED and
ROLLED by the query offset so the same SPMD program works on every
core (softmax+PV are permutation-invariant along the key axis).

Device layout (everything feature-on-partition, token-on-free):
  kT [640, 4096] (5 chunks of 128 = 2 heads each)  "scoresT" = K Q^T
  v  [4096, 650] (65-stride per head: 64 cols + ones col -> softmax
     denominators fall out of the PV matmul as PSUM row 64)
  probs: scoresT in PSUM -> ScalarE exp -> bf16 SBUF (ScalarE is the
     bottleneck engine: S*S*BH/8 = 41.9M exps/core)
  normalization: reciprocal of denom row, rank-1 PE outer product to
     broadcast across partitions (DVE cannot cross partitions), DVE mult.
All matmuls bf16 (f32 PSUM accumulation).
"""

import sys

if "/opt/trn_rl_repo" not in sys.path:
    sys.path.insert(0, "/opt/trn_rl_repo")

from contextlib import ExitStack

import ml_dtypes
import numpy as np

import concourse.bass as bass
import concourse.tile as tile
from concourse import mybir
from concourse.bass import ts

BF16 = mybir.dt.bfloat16
F32 = mybir.dt.float32

B, S, C = 2, 4096, 640
H, DH = 10, 64
NCORES = 8
GROUP = 4  # cores per batch element
SQ = S // GROUP  # 1024 queries per core
SCALE = 0.125  # 1/sqrt(64)
CCH = C // 128  # 5 feature chunks (2 heads each)
NJT = S // 512  # 8 key tiles for K proj
NJC = S // 128  # 32 key chunks for attention
NIT = SQ // 512  # 2 query tiles
VST = DH + 1  # 65: per-head stride in v tiles (ones col appended)

# exp group structure: j-chunks per ScalarE activation call.  Groups of 2
# (not 3) cost ~12us extra ScalarE call overhead but shrink the score
# double-buffer from 6 PSUM banks to 4, freeing 2 banks for a background
# pool that lets K/Q/V projections and the it0 output projection overlap
# the attention phase instead of running with ScalarE idle.
EXP_GROUPS = [list(range(g, min(g + 2, NJC))) for g in range(0, NJC, 2)]


def build_nc() -> bass.Bass:
    nc = bass.Bass()
    hsT = nc.declare_dram_parameter("hsT", [C, S], BF16, isOutput=False)
    res = nc.declare_dram_parameter("res", [C, SQ], F32, isOutput=False)
    wqT = nc.declare_dram_parameter("wqT", [C, C], BF16, isOutput=False)
    wkT = nc.declare_dram_parameter("wkT", [C, C], BF16, isOutput=False)
    wvT = nc.declare_dram_parameter("wvT", [C, C], BF16, isOutput=False)
    woT = nc.declare_dram_parameter("woT", [C, C], BF16, isOutput=False)
    out = nc.declare_dram_parameter("out", [C, SQ], F32, isOutput=True)

    with ExitStack() as ctx:
        tc = ctx.enter_context(tile.TileContext(nc))
        # outer pool: tensors whose lifetime spans projections AND attention
        sb = ctx.enter_context(tc.tile_pool(name="sb", bufs=1))

        kT_sb = [sb.tile([128, S], BF16, tag=f"kT{i}", name=f"kT{i}") for i in range(CCH)]
        # per-head q, zero-padded to full 128-row contraction: partial-K
        # (K=64) matmuls keep the PE HAM-throttled at 1.2 GHz -- padding the
        # contraction with zero rows is exact and runs at the warm rate.
        qTz_sb = [
            [sb.tile([128, SQ], BF16, tag=f"qz{i}_{p}", name=f"qz{i}_{p}")
             for p in range(2)]
            for i in range(CCH)
        ]
        v_sb = [sb.tile([128, H * VST], BF16, tag=f"v{j}", name=f"v{j}") for j in range(NJC)]
        ones_sb = sb.tile([128, DH], BF16, tag="ones", name="ones")
        nc.vector.memset(ones_sb[:], 1.0)

        # ---------------- load + first projections ----------------
        # Only kT/qTz for head pair 0 are produced up front; the remaining
        # K/Q chunks, the whole V projection, and the it0 output projection
        # are emitted as background work inside the attention loop (2 spare
        # PSUM banks, tag "pp"), so ScalarE starts ~90us earlier.
        load = ctx.enter_context(tc.tile_pool(name="load", bufs=1))
        hsT_sb = []
        wq_sb, wk_sb, wv_sb = [], [], []
        for i in range(CCH):
            w = load.tile([128, C], BF16, tag=f"wk{i}", name=f"wk{i}")
            nc.sync.dma_start(w[:], wkT[ts(i, 128), :])
            wk_sb.append(w)
            t = load.tile([128, S], BF16, tag=f"hsT{i}", name=f"hsT{i}")
            nc.sync.dma_start(t[:, 0:SQ], hsT[ts(i, 128), 0:SQ])
            hsT_sb.append(t)
        for i in range(CCH):
            for name, lst, srcp in (("wq", wq_sb, wqT), ("wv", wv_sb, wvT)):
                w = load.tile([128, C], BF16, tag=f"{name}{i}", name=f"{name}{i}")
                nc.sync.dma_start(w[:], srcp[ts(i, 128), :])
                lst.append(w)

        def emit_hsT_tail():
            # deferred until after the first exp so ScalarE's conservative
            # vector-clock waits don't cover this 4MB of DMA
            for blk in range(SQ, S, SQ):
                for i in range(CCH):
                    nc.sync.dma_start(
                        hsT_sb[i][:, blk : blk + SQ],
                        hsT[ts(i, 128), blk : blk + SQ],
                    )

        def emit_kproj(dc, jt, pool):
            ps = pool.tile([128, 512], F32, tag="pp", name="pp", bufs=2)
            for cc in range(CCH):
                nc.tensor.matmul(
                    ps[:],
                    wk_sb[cc][:, ts(dc, 128)],
                    hsT_sb[cc][:, ts(jt, 512)],
                    start=(cc == 0),
                    stop=(cc == CCH - 1),
                )
            nc.vector.tensor_copy(kT_sb[dc][:, ts(jt, 512)], ps[:])

        def emit_qproj(dc, it, pool):
            ps = pool.tile([128, 512], F32, tag="pp", name="pp", bufs=2)
            for cc in range(CCH):
                nc.tensor.matmul(
                    ps[:],
                    wq_sb[cc][:, ts(dc, 128)],
                    hsT_sb[cc][:, ts(it, 512)],
                    start=(cc == 0),
                    stop=(cc == CCH - 1),
                )
            nc.vector.tensor_copy(qTz_sb[dc][0][0:DH, ts(it, 512)], ps[0:DH, :])
            nc.vector.tensor_copy(qTz_sb[dc][1][DH:128, ts(it, 512)], ps[DH:128, :])

        def emit_vproj(jc, pool):
            vt = v_sb[jc]
            v3 = vt[:].rearrange("p (h x) -> p h x", x=VST)
            nc.vector.memset(v3[:, :, DH : DH + 1], 1.0)
            for d0, dn in ((0, 512), (512, 128)):
                ps = pool.tile([128, 512], F32, tag="pp", name="pp", bufs=2)
                for cc in range(CCH):
                    nc.tensor.matmul(
                        ps[:, 0:dn],
                        hsT_sb[cc][:, ts(jc, 128)],
                        wv_sb[cc][:, d0 : d0 + dn],
                        start=(cc == 0),
                        stop=(cc == CCH - 1),
                    )
                nc.vector.tensor_copy(
                    v3[:, d0 // DH : (d0 + dn) // DH, 0:DH],
                    ps[:, 0:dn].rearrange("p (h x) -> p h x", x=DH),
                )

        with tc.tile_pool(name="pp0", bufs=2, space="PSUM") as pp0:
            for dc in range(CCH):
                nc.vector.memset(qTz_sb[dc][0][DH:128, :], 0.0)
                nc.vector.memset(qTz_sb[dc][1][0:DH, :], 0.0)
            for jt in range(2):
                emit_kproj(0, jt, pp0)
            for it in range(NIT):
                emit_qproj(0, it, pp0)

        # ---------------- attention phase ----------------
        attn_sb = [sb.tile([128, SQ], BF16, tag=f"attn{h}", name=f"attn{h}")
                   for h in range(H)]
        for h in range(H):
            nc.vector.memset(attn_sb[h][DH:128, :], 0.0)
        with tc.tile_pool(name="ap", bufs=1, space="PSUM") as ap, \
             tc.tile_pool(name="pt", bufs=4) as pt_pool, \
             tc.tile_pool(name="ob", bufs=3) as ob, \
             tc.tile_pool(name="scratch", bufs=3) as scratch:
            def norm_dve(h, pv, p_isl):
                recip = scratch.tile([DH + 1, 512], BF16, tag="recip",
                                     name="recip")
                with nc.allow_low_precision(reason="softmax recip bf16"):
                    nc.vector.reciprocal(recip[DH : DH + 1, :],
                                         pv[DH : DH + 1, :])
                raw = scratch.tile([DH, 512], BF16, tag="raw", name="raw")
                nc.vector.tensor_copy(raw[:], pv[0:DH, :])
                return (h, pv, p_isl, recip, raw)

            def norm_pe(state, anchor):
                # rank-1 PE broadcast of the reciprocal, pinned behind the
                # anchor QK so the underpriced 3.4us reciprocal is hidden
                h, pv, p_isl, recip, raw = state
                r_mm = nc.tensor.matmul(
                    pv[0:DH, :],
                    ones_sb[DH : DH + 1, :],
                    recip[DH : DH + 1, :],
                    start=True,
                    stop=True,
                )
                if anchor is not None:
                    tile.add_dep_helper(
                        r_mm.ins, anchor.ins, sync=False,
                        reason="norm R after anchor QK (hide recip latency)",
                    )
                nc.vector.tensor_mul(
                    attn_sb[h][0:DH, p_isl], raw[:], pv[0:DH, :]
                )

            def emit_oproj(ec, it):
                wos = []
                for h in range(H):
                    wt = ob.tile([128, 128], BF16, tag="woec", name="woec",
                                 bufs=12)
                    nc.vector.memset(wt[DH:128, :], 0.0)
                    nc.sync.dma_start(wt[0:DH, :], woT[ts(h, DH), ts(ec, 128)])
                    wos.append(wt)
                ps = ap.tile([128, 512], F32, tag="pp", name="pp", bufs=2)
                for h in range(H):
                    nc.tensor.matmul(
                        ps[:],
                        wos[h][:],
                        attn_sb[h][:, ts(it, 512)],
                        start=(h == 0),
                        stop=(h == H - 1),
                    )
                rt = ob.tile([128, 512], F32, tag="rt", name="rt", bufs=2)
                nc.sync.dma_start(rt[:], res[ts(ec, 128), ts(it, 512)])
                ot = ob.tile([128, 512], F32, tag="ot", name="ot", bufs=2)
                nc.vector.tensor_add(ot[:], ps[:], rt[:])
                nc.sync.dma_start(out[ts(ec, 128), ts(it, 512)], ot[:])

            pending = None
            bg = [
                (lambda jt=jt: emit_kproj(0, jt, ap)) for jt in range(2, NJT)
            ]
            for it in range(NIT):
                isl = ts(it, 512)
                for hp in range(CCH):
                    if it == 0 and hp + 1 < CCH:
                        # prefetch next pair's K/Q chunks in the background
                        bg.extend(
                            (lambda jt=jt, dc=hp + 1: emit_kproj(dc, jt, ap))
                            for jt in range(NJT)
                        )
                        bg.extend(
                            (lambda q_it=q_it, dc=hp + 1: emit_qproj(dc, q_it, ap))
                            for q_it in range(NIT)
                        )
                    if it == 1 and hp == 1:
                        bg.extend(
                            (lambda ec=ec: emit_oproj(ec, 0)) for ec in range(CCH)
                        )
                    for h in (2 * hp, 2 * hp + 1):
                        pv = ap.tile([DH + 1, 512], F32, tag="pv", bufs=2,
                                     name="pv")
                        for gi, g in enumerate(EXP_GROUPS):
                            sc = ap.tile([128, 512 * len(g)], F32, tag="sc",
                                         bufs=2, name="sc")
                            last_qk = None
                            for k, jc in enumerate(g):
                                last_qk = nc.tensor.matmul(
                                    sc[:, ts(k, 512)],
                                    kT_sb[hp][:, ts(jc, 128)],
                                    qTz_sb[hp][h % 2][:, isl],
                                    start=True,
                                    stop=True,
                                )
                            pt = pt_pool.tile([128, 512 * len(g)], BF16,
                                              tag="pt", name="pt")
                            nc.scalar.activation(
                                pt[:], sc[:], mybir.ActivationFunctionType.Exp,
                                bias=0.0, scale=SCALE,
                            )
                            if pending is not None and gi == 5:
                                norm_pe(pending, last_qk)
                                pending = None
                            if it == 0 and hp == 0 and h == 0:
                                if gi == 0:
                                    emit_hsT_tail()
                                # V projection rides inside the first head's
                                # window, each chunk just ahead of its PV;
                                # remaining kT dc0 stripes drain alongside
                                if bg:
                                    bg.pop(0)()
                                for jc in g:
                                    emit_vproj(jc, ap)
                            elif bg:
                                bg.pop(0)()
                            for k, jc in enumerate(g):
                                nc.tensor.matmul(
                                    pv[:],
                                    v_sb[jc][:, h * VST : (h + 1) * VST],
                                    pt[:, ts(k, 512)],
                                    start=(jc == 0),
                                    stop=(jc == NJC - 1),
                                )
                        pending = norm_dve(h, pv, isl)
                    # barrier: next pair's kT/qTz must be fully emitted
                    # before its first QK reads them
                    while bg:
                        bg.pop(0)()
            norm_pe(pending, None)
            for ec in range(CCH):
                emit_oproj(ec, 1)

    _spill_matmul_waits(nc)
    return nc


# walrus embedded-sync-wait capacity per BIR opcode.  Matmult holds a
# single wait; excess waits hoist onto the paired Ldweights (in-order
# issue on PE makes that equivalent).  Other compute ops spill onto
# EventSemaphore carrier instructions inserted just before them on the
# same engine.  DMACopy / Drain / EventSemaphore handle many waits
# natively (bacc emits such itself) and are left alone.
_WAIT_CAPS = {
    "InstMatmult": 1,
    "InstLdweights": 1,
    "InstActivation": 1,
    "InstReciprocal": 1,
    "InstTensorTensor": 1,
    "InstTensorCopy": 1,
    "InstTensorScalarPtr": 1,
    "InstTensorReduce": 1,
    "InstMemset": 1,
    "InstDMACopy": 1,
    "InstDrain": 1,
    "InstCustomDveAnt": 1,
}
_ES_CAP = 2  # waits per EventSemaphore carrier (walrus: <=2 waits, <=1 update)


def _spill_matmul_waits(nc: bass.Bass) -> None:
    spill_id = [0]

    def carriers(excess, engine):
        out = []
        for i in range(0, len(excess), _ES_CAP):
            es = mybir.InstEventSemaphore(
                name=f"wait-spill-{spill_id[0]}", ins=[], outs=[]
            )
            spill_id[0] += 1
            es.engine = engine
            es.sync_info = mybir.SyncInfo(
                on_wait=excess[i : i + _ES_CAP], on_update=[]
            )
            out.append(es)
        return out

    for f in nc.m.functions:
        for blk in f.blocks:
            insts = blk.instructions
            i = 0
            while i < len(insts):
                inst = insts[i]
                tn = type(inst).__name__
                cap = _WAIT_CAPS.get(tn)
                si = inst.sync_info
                if cap is None or si is None or len(si.on_wait) <= cap:
                    i += 1
                    continue
                w = list(si.on_wait)
                if tn == "InstMatmult" and cap == 1:
                    # Keep the latest-satisfied dependency (the ACT-produced
                    # operand, e.g. probs from exp) embedded on the matmul and
                    # hoist early ones onto the Ldweights: a wait on the LDW
                    # blocks its background prefetch and serializes ~50ns of
                    # weight-load into every PV matmul.
                    acts = [x for x in w if "Activation" in (x.ant_name or "")]
                    if acts:
                        keep = [acts[-1]]
                        excess = [x for x in w if x is not acts[-1]]
                    else:
                        keep, excess = w[-cap:], w[:-cap]
                else:
                    keep, excess = w[-cap:], w[:-cap]
                prev = insts[i - 1] if i > 0 else None
                if (
                    tn == "InstMatmult"
                    and prev is not None
                    and type(prev).__name__ == "InstLdweights"
                    and len(((prev.sync_info and prev.sync_info.on_wait) or []))
                    + len(excess) <= 1
                ):
                    psi = prev.sync_info
                    pw = list(psi.on_wait) if psi is not None else []
                    pu = list(psi.on_update) if psi is not None else []
                    prev.sync_info = mybir.SyncInfo(on_wait=pw + excess, on_update=pu)
                else:
                    new = carriers(excess, inst.engine)
                    insts[i:i] = new
                    i += len(new)
                inst.sync_info = mybir.SyncInfo(
                    on_wait=keep, on_update=list(si.on_update)
                )
                i += 1


_CACHED_NC = None


def get_nc() -> bass.Bass:
    global _CACHED_NC
    if _CACHED_NC is None:
        _CACHED_NC = build_nc()
    return _CACHED_NC


def make_in_maps(hidden_states, Wq, Wk, Wv, Wo, b_out):
    hs = np.asarray(hidden_states, dtype=np.float32)
    bf = ml_dtypes.bfloat16
    wqT = np.ascontiguousarray(np.asarray(Wq, np.float32).T).astype(bf)
    wkT = np.ascontiguousarray(np.asarray(Wk, np.float32).T).astype(bf)
    wvT = np.ascontiguousarray(np.asarray(Wv, np.float32).T).astype(bf)
    woT = np.ascontiguousarray(np.asarray(Wo, np.float32).T).astype(bf)
    bias = np.asarray(b_out, np.float32).reshape(C, 1)
    in_maps = []
    for c in range(NCORES):
        b, g = divmod(c, GROUP)
        i0 = g * SQ
        hsTb = hs[b].T  # [C, S]
        in_maps.append(
            {
                "hsT": np.ascontiguousarray(np.roll(hsTb, -i0, axis=1)).astype(bf),
                "res": np.ascontiguousarray(hsTb[:, i0 : i0 + SQ]) + bias,
                "wqT": wqT,
                "wkT": wkT,
                "wvT": wvT,
                "woT": woT,
            }
        )
    return in_maps


def assemble(results) -> np.ndarray:
    y = np.empty((B, S, C), np.float32)
    for c in range(NCORES):
        b, g = divmod(c, GROUP)
        i0 = g * SQ
        y[b, i0 : i0 + SQ, :] = np.asarray(results[c]["out"], np.float32).T
    return y


def kernel(**inputs) -> np.ndarray:
    from concourse.bass_utils import run_bass_kernel_spmd

    nc = get_nc()
    in_maps = make_in_maps(**inputs)
    res = run_bass_kernel_spmd(nc, in_maps, list(range(NCORES)))
    return assemble(res.results)


if __name__ == "__main__":
    import reference

    inputs = {k: np.asarray(v) for k, v in reference.setup_inputs().items()}
    got = kernel(**inputs)
    want = np.asarray(reference.reference(**inputs))
    err = np.linalg.norm(got - want) / np.linalg.norm(want)
    print("Relative error:", err)



# revision 8
# speedup vs baseline: 1.0181x; 1.0181x over previous
"""Multi-head attention (AttnProcessor2_0) on 8 TRN2 NeuronCores.

Problem: B=2, S=4096, C=640, H=10, Dh=64.
  q/k/v = hs @ W{q,k,v}.T ; per-head scores = q k^T / 8 ; softmax ;
  out = probs v ; y = out @ Wo.T + b_out + hs

Sharding (no collectives): core c -> batch b=c//4, query block g=c%4
(1024 queries).  Each core recomputes full K/V for its batch (head-dim
on partitions), computes its own S/4 x S attention block, output
projection, bias+residual.  Host passes hidden states TRANSPOSED and
ROLLED by the query offset so the same SPMD program works on every
core (softmax+PV are permutation-invariant along the key axis).

Device layout (everything feature-on-partition, token-on-free):
  kT [640, 4096] (5 chunks of 128 = 2 heads each)  "scoresT" = K Q^T
  qT [128, 1024] per head pair (rows 0:64 head even, 64:128 head odd)
  v  [4096, 650] (65-stride per head: 64 cols + ones col -> softmax
     denominators fall out of the PV matmul as PSUM row 64)
  QK: both heads of a pair run CONCURRENTLY as K=64 row-tiled matmuls
     (tile_position (0,0) and (64,0)) writing adjacent PSUM banks --
     2x the padded-contraction QK throughput of the old layout.
  probs: scoresT in PSUM -> ScalarE exp -> bf16 SBUF
  normalization: pv [65,512] copied to SBUF right after the PV
     accumulation stops (frees the PSUM bank), reciprocal of denom row,
     rank-1 PE outer product into a scratch PSUM bank to broadcast
     across partitions, DVE mult.
All matmuls bf16 (f32 PSUM accumulation).
"""

import sys

if "/opt/trn_rl_repo" not in sys.path:
    sys.path.insert(0, "/opt/trn_rl_repo")

from contextlib import ExitStack

import ml_dtypes
import numpy as np

import concourse.bass as bass
import concourse.tile as tile
from concourse import mybir
from concourse.bass import ts

BF16 = mybir.dt.bfloat16
F32 = mybir.dt.float32

B, S, C = 2, 4096, 640
H, DH = 10, 64
NCORES = 8
GROUP = 4  # cores per batch element
SQ = S // GROUP  # 1024 queries per core
SCALE = 0.125  # 1/sqrt(64)
CCH = C // 128  # 5 feature chunks (2 heads each)
NJT = S // 512  # 8 key tiles for K proj
NJC = S // 128  # 32 key chunks for attention
NIT = SQ // 512  # 2 query tiles
VST = DH + 1  # 65: per-head stride in v tiles (ones col appended)

# Schraudolph exp offload: selected score chunks compute exp on the DVE
# as a bf16 bit-trick (one tensor_scalar: bits = round(s*A + B) viewed as
# bf16 gives 2^(s*log2e) with ~2% per-element jitter and ~zero mean; any
# constant bias cancels in the softmax ratio).  This moves work off the
# bottleneck ScalarE onto the (slack) DVE.  Set empty to disable.
SCHRAUD_A = SCALE * 128.0 / float(np.log(2.0))  # folds the 1/8 score scale
SCHRAUD_B = 127.0 * 128.0 - 7.45
OFFLOAD_JC = frozenset()


def build_nc() -> bass.Bass:
    nc = bass.Bass()
    hsT = nc.declare_dram_parameter("hsT", [C, S], BF16, isOutput=False)
    res = nc.declare_dram_parameter("res", [C, SQ], F32, isOutput=False)
    wqT = nc.declare_dram_parameter("wqT", [C, C], BF16, isOutput=False)
    wkT = nc.declare_dram_parameter("wkT", [C, C], BF16, isOutput=False)
    wvT = nc.declare_dram_parameter("wvT", [C, C], BF16, isOutput=False)
    woT = nc.declare_dram_parameter("woT", [C, C], BF16, isOutput=False)
    out = nc.declare_dram_parameter("out", [C, SQ], F32, isOutput=True)

    with ExitStack() as ctx:
        tc = ctx.enter_context(tile.TileContext(nc))
        # outer pool: tensors whose lifetime spans projections AND attention
        sb = ctx.enter_context(tc.tile_pool(name="sb", bufs=1))

        kT_sb = [sb.tile([128, S], BF16, tag=f"kT{i}", name=f"kT{i}") for i in range(CCH)]
        # head-pair q: rows 0:64 = even head, 64:128 = odd head.  The QK
        # matmuls are K=64 row-tiled (tile_position (0,0)/(64,0)) and run
        # concurrently in the PE array -- no zero padding needed.
        qT_sb = [sb.tile([128, SQ], BF16, tag=f"qT{i}", name=f"qT{i}") for i in range(CCH)]
        v_sb = [sb.tile([128, H * VST], BF16, tag=f"v{j}", name=f"v{j}") for j in range(NJC)]
        ones_sb = sb.tile([128, DH], BF16, tag="ones", name="ones")
        nc.vector.memset(ones_sb[:], 1.0)

        # ---------------- load + first projections ----------------
        # Only kT/qT for head pair 0 (keys 0:1024) are produced up front;
        # the remaining K/Q chunks, the whole V projection, and the it0
        # output projection are emitted as background work inside the
        # attention loop (2 spare PSUM banks, tag "pp").
        load = ctx.enter_context(tc.tile_pool(name="load", bufs=1))
        hsT_sb = []
        wq_sb, wk_sb, wv_sb = [], [], []
        for i in range(CCH):
            w = load.tile([128, C], BF16, tag=f"wk{i}", name=f"wk{i}")
            nc.sync.dma_start(w[:], wkT[ts(i, 128), :])
            wk_sb.append(w)
            t = load.tile([128, S], BF16, tag=f"hsT{i}", name=f"hsT{i}")
            nc.sync.dma_start(t[:, 0:SQ], hsT[ts(i, 128), 0:SQ])
            hsT_sb.append(t)
        for i in range(CCH):
            for name, lst, srcp in (("wq", wq_sb, wqT), ("wv", wv_sb, wvT)):
                w = load.tile([128, C], BF16, tag=f"{name}{i}", name=f"{name}{i}")
                nc.sync.dma_start(w[:], srcp[ts(i, 128), :])
                lst.append(w)

        def emit_hsT_tail():
            # deferred until after the first exp so ScalarE's conservative
            # vector-clock waits don't cover this 4MB of DMA
            for blk in range(SQ, S, SQ):
                for i in range(CCH):
                    nc.sync.dma_start(
                        hsT_sb[i][:, blk : blk + SQ],
                        hsT[ts(i, 128), blk : blk + SQ],
                    )

        def emit_kproj(dc, jt, pool):
            ps = pool.tile([128, 512], F32, tag="pp", name="pp", bufs=2)
            for cc in range(CCH):
                nc.tensor.matmul(
                    ps[:],
                    wk_sb[cc][:, ts(dc, 128)],
                    hsT_sb[cc][:, ts(jt, 512)],
                    start=(cc == 0),
                    stop=(cc == CCH - 1),
                )
            nc.vector.tensor_copy(kT_sb[dc][:, ts(jt, 512)], ps[:])

        def emit_qproj(dc, it, pool):
            ps = pool.tile([128, 512], F32, tag="pp", name="pp", bufs=2)
            for cc in range(CCH):
                nc.tensor.matmul(
                    ps[:],
                    wq_sb[cc][:, ts(dc, 128)],
                    hsT_sb[cc][:, ts(it, 512)],
                    start=(cc == 0),
                    stop=(cc == CCH - 1),
                )
            nc.vector.tensor_copy(qT_sb[dc][:, ts(it, 512)], ps[:])

        def emit_vproj(jc, pool):
            vt = v_sb[jc]
            v3 = vt[:].rearrange("p (h x) -> p h x", x=VST)
            nc.vector.memset(v3[:, :, DH : DH + 1], 1.0)
            for d0, dn in ((0, 512), (512, 128)):
                ps = pool.tile([128, 512], F32, tag="pp", name="pp", bufs=2)
                for cc in range(CCH):
                    nc.tensor.matmul(
                        ps[:, 0:dn],
                        hsT_sb[cc][:, ts(jc, 128)],
                        wv_sb[cc][:, d0 : d0 + dn],
                        start=(cc == 0),
                        stop=(cc == CCH - 1),
                    )
                nc.vector.tensor_copy(
                    v3[:, d0 // DH : (d0 + dn) // DH, 0:DH],
                    ps[:, 0:dn].rearrange("p (h x) -> p h x", x=DH),
                )

        with tc.tile_pool(name="pp0", bufs=2, space="PSUM") as pp0:
            for jt in range(2):
                emit_kproj(0, jt, pp0)
            emit_qproj(0, 0, pp0)

        # ---------------- attention phase ----------------
        attn_sb = [sb.tile([128, SQ], BF16, tag=f"attn{h}", name=f"attn{h}")
                   for h in range(H)]
        for h in range(H):
            nc.vector.memset(attn_sb[h][DH:128, :], 0.0)
        with tc.tile_pool(name="ap", bufs=1, space="PSUM") as ap, \
             tc.tile_pool(name="pt", bufs=4) as pt_pool, \
             tc.tile_pool(name="ob", bufs=3) as ob, \
             tc.tile_pool(name="scratch", bufs=3) as scratch:
            def norm_dve(h, pv, p_isl):
                # one copy drains the whole pv accumulator (incl. denom
                # row) to SBUF, freeing the PSUM bank immediately
                raw = scratch.tile([DH + 1, 512], BF16, tag="raw", name="raw",
                                   bufs=4)
                nc.vector.tensor_copy(raw[:], pv[:])
                rc = scratch.tile([DH + 1, 512], BF16, tag="rc", name="rc",
                                  bufs=4)
                with nc.allow_low_precision(reason="softmax recip bf16"):
                    nc.vector.reciprocal(rc[DH : DH + 1, :],
                                         raw[DH : DH + 1, :])
                return (h, p_isl, rc, raw)

            def norm_pe(state, anchor, pool):
                # rank-1 PE broadcast of the reciprocal, pinned behind the
                # anchor QK so the slow DVE reciprocal is hidden
                h, p_isl, rc, raw = state
                ps = pool.tile([128, 512], F32, tag="pp", name="pp", bufs=2)
                r_mm = nc.tensor.matmul(
                    ps[0:DH, :],
                    ones_sb[DH : DH + 1, 0:DH],
                    rc[DH : DH + 1, :],
                    start=True,
                    stop=True,
                )
                if anchor is not None:
                    tile.add_dep_helper(
                        r_mm.ins, anchor.ins, sync=False,
                        reason="norm R after anchor QK (hide recip latency)",
                    )
                nc.vector.tensor_mul(
                    attn_sb[h][0:DH, p_isl], raw[0:DH, :], ps[0:DH, :]
                )

            def emit_oproj(ec, it):
                wos = []
                for h in range(H):
                    wt = ob.tile([128, 128], BF16, tag="woec", name="woec",
                                 bufs=12)
                    nc.vector.memset(wt[DH:128, :], 0.0)
                    nc.sync.dma_start(wt[0:DH, :], woT[ts(h, DH), ts(ec, 128)])
                    wos.append(wt)
                ps = ap.tile([128, 512], F32, tag="pp", name="pp", bufs=2)
                for h in range(H):
                    nc.tensor.matmul(
                        ps[:],
                        wos[h][:],
                        attn_sb[h][:, ts(it, 512)],
                        start=(h == 0),
                        stop=(h == H - 1),
                    )
                rt = ob.tile([128, 512], F32, tag="rt", name="rt", bufs=2)
                nc.sync.dma_start(rt[:], res[ts(ec, 128), ts(it, 512)])
                ot = ob.tile([128, 512], F32, tag="ot", name="ot", bufs=2)
                nc.vector.tensor_add(ot[:], ps[:], rt[:])
                nc.sync.dma_start(out[ts(ec, 128), ts(it, 512)], ot[:])

            # Background-work schedule: each window (it, hp) carries a
            # jc -> [callables] map.  Self-feeding kT stripes (jt2..7 of
            # this window's own chunk) land just ahead of their first QK
            # use; the next pair's kT jt0/jt1 + qT it-slice land at the
            # window's end; it1 windows carry the it0 output projections
            # and the lazily-deferred it1 q-projections.
            SELF_JC = (5, 9, 13, 17, 21, 25)

            def mk_todo(it, hp):
                todo = {}

                def add(jc, fn):
                    todo.setdefault(jc, []).append(fn)

                if it == 0:
                    for j, jt in zip(SELF_JC, range(2, NJT)):
                        add(j, lambda dc=hp, jt=jt: emit_kproj(dc, jt, ap))
                    if hp == 0:
                        # V chunks 0..7 front-loaded (keys 0:1024 resident
                        # before the hsT tail lands), then one chunk per
                        # step four steps ahead of its PV use
                        for jc in range(4):
                            add(jc, lambda j=2 * jc: emit_vproj(j, ap))
                            add(jc, lambda j=2 * jc + 1: emit_vproj(j, ap))
                        for jc in range(4, 28):
                            add(jc, lambda j=jc + 4: emit_vproj(j, ap))
                    if hp < CCH - 1:
                        add(27, lambda dc=hp + 1: emit_kproj(dc, 0, ap))
                        add(29, lambda dc=hp + 1: emit_kproj(dc, 1, ap))
                        add(31, lambda dc=hp + 1: emit_qproj(dc, 0, ap))
                    else:
                        add(27, lambda: emit_qproj(0, 1, ap))
                else:
                    if hp == 0:
                        add(27, lambda: emit_qproj(1, 1, ap))
                    elif hp == 1:
                        add(5, lambda: emit_oproj(0, 0))
                        add(13, lambda: emit_oproj(1, 0))
                        add(27, lambda: emit_qproj(2, 1, ap))
                    elif hp == 2:
                        add(5, lambda: emit_oproj(2, 0))
                        add(27, lambda: emit_qproj(3, 1, ap))
                    elif hp == 3:
                        add(5, lambda: emit_oproj(3, 0))
                        add(27, lambda: emit_qproj(4, 1, ap))
                    else:
                        add(5, lambda: emit_oproj(4, 0))
                return todo

            pending = []
            for it in range(NIT):
                isl = ts(it, 512)
                for hp in range(CCH):
                    h0, h1 = 2 * hp, 2 * hp + 1
                    todo = mk_todo(it, hp)
                    pv0 = ap.tile([DH + 1, 512], F32, tag="pv0", bufs=1,
                                  name="pv0")
                    pv1 = ap.tile([DH + 1, 512], F32, tag="pv1", bufs=1,
                                  name="pv1")
                    for jc in range(NJC):
                        sc = ap.tile([128, 1024], F32, tag="sc", bufs=2,
                                     name="sc")
                        qk0 = nc.tensor.matmul(
                            sc[:, 0:512],
                            kT_sb[hp][0:DH, ts(jc, 128)],
                            qT_sb[hp][0:DH, isl],
                            start=True,
                            stop=True,
                        )
                        nc.tensor.matmul(
                            sc[:, 512:1024],
                            kT_sb[hp][DH:128, ts(jc, 128)],
                            qT_sb[hp][DH:128, isl],
                            start=True,
                            stop=True,
                        )
                        pt = pt_pool.tile([128, 1024], BF16,
                                          tag="pt", name="pt")
                        if (it, hp) != (0, 0) and jc in OFFLOAD_JC:
                            nc.vector.tensor_scalar(
                                out=pt[:].bitcast(mybir.dt.int16),
                                in0=sc[:],
                                scalar1=SCHRAUD_A,
                                scalar2=SCHRAUD_B,
                                op0=mybir.AluOpType.mult,
                                op1=mybir.AluOpType.add,
                            )
                        else:
                            nc.scalar.activation(
                                pt[:], sc[:],
                                mybir.ActivationFunctionType.Exp,
                                bias=0.0, scale=SCALE,
                            )
                        if it == 0 and hp == 0 and jc == 0:
                            emit_hsT_tail()
                        if pending and jc in (5, 7):
                            norm_pe(pending.pop(0), qk0, ap)
                        for fn in todo.get(jc, ()):
                            fn()
                        nc.tensor.matmul(
                            pv0[:],
                            v_sb[jc][:, h0 * VST : (h0 + 1) * VST],
                            pt[:, 0:512],
                            start=(jc == 0),
                            stop=(jc == NJC - 1),
                        )
                        nc.tensor.matmul(
                            pv1[:],
                            v_sb[jc][:, h1 * VST : (h1 + 1) * VST],
                            pt[:, 512:1024],
                            start=(jc == 0),
                            stop=(jc == NJC - 1),
                        )
                    pending.append(norm_dve(h0, pv0, isl))
                    pending.append(norm_dve(h1, pv1, isl))
            for st in pending:
                norm_pe(st, None, ap)
            for ec in range(CCH):
                emit_oproj(ec, 1)

    _spill_matmul_waits(nc)
    return nc


# walrus embedded-sync-wait capacity per BIR opcode.  Matmult holds a
# single wait; excess waits hoist onto the paired Ldweights (in-order
# issue on PE makes that equivalent).  Other compute ops spill onto
# EventSemaphore carrier instructions inserted just before them on the
# same engine.  DMACopy / Drain / EventSemaphore handle many waits
# natively (bacc emits such itself) and are left alone.
_WAIT_CAPS = {
    "InstMatmult": 1,
    "InstLdweights": 1,
    "InstActivation": 1,
    "InstReciprocal": 1,
    "InstTensorTensor": 1,
    "InstTensorCopy": 1,
    "InstTensorScalarPtr": 1,
    "InstTensorReduce": 1,
    "InstMemset": 1,
    "InstDMACopy": 1,
    "InstDrain": 1,
    "InstCustomDveAnt": 1,
}
_ES_CAP = 2  # waits per EventSemaphore carrier (walrus: <=2 waits, <=1 update)


def _spill_matmul_waits(nc: bass.Bass) -> None:
    spill_id = [0]

    def carriers(excess, engine):
        out = []
        for i in range(0, len(excess), _ES_CAP):
            es = mybir.InstEventSemaphore(
                name=f"wait-spill-{spill_id[0]}", ins=[], outs=[]
            )
            spill_id[0] += 1
            es.engine = engine
            es.sync_info = mybir.SyncInfo(
                on_wait=excess[i : i + _ES_CAP], on_update=[]
            )
            out.append(es)
        return out

    for f in nc.m.functions:
        for blk in f.blocks:
            insts = blk.instructions
            i = 0
            while i < len(insts):
                inst = insts[i]
                tn = type(inst).__name__
                cap = _WAIT_CAPS.get(tn)
                si = inst.sync_info
                if cap is None or si is None or len(si.on_wait) <= cap:
                    i += 1
                    continue
                w = list(si.on_wait)
                if tn == "InstMatmult" and cap == 1:
                    # Keep the latest-satisfied dependency (the ACT-produced
                    # operand, e.g. probs from exp) embedded on the matmul and
                    # hoist early ones onto the Ldweights: a wait on the LDW
                    # blocks its background prefetch and serializes ~50ns of
                    # weight-load into every PV matmul.
                    acts = [x for x in w if "Activation" in (x.ant_name or "")]
                    if acts:
                        keep = [acts[-1]]
                        excess = [x for x in w if x is not acts[-1]]
                    else:
                        keep, excess = w[-cap:], w[:-cap]
                else:
                    keep, excess = w[-cap:], w[:-cap]
                prev = insts[i - 1] if i > 0 else None
                if (
                    tn == "InstMatmult"
                    and prev is not None
                    and type(prev).__name__ == "InstLdweights"
                    and len(((prev.sync_info and prev.sync_info.on_wait) or []))
                    + len(excess) <= 1
                ):
                    psi = prev.sync_info
                    pw = list(psi.on_wait) if psi is not None else []
                    pu = list(psi.on_update) if psi is not None else []
                    prev.sync_info = mybir.SyncInfo(on_wait=pw + excess, on_update=pu)
                else:
                    new = carriers(excess, inst.engine)
                    insts[i:i] = new
                    i += len(new)
                inst.sync_info = mybir.SyncInfo(
                    on_wait=keep, on_update=list(si.on_update)
                )
                i += 1


_CACHED_NC = None


def get_nc() -> bass.Bass:
    global _CACHED_NC
    if _CACHED_NC is None:
        _CACHED_NC = build_nc()
    return _CACHED_NC


def make_in_maps(hidden_states, Wq, Wk, Wv, Wo, b_out):
    hs = np.asarray(hidden_states, dtype=np.float32)
    bf = ml_dtypes.bfloat16
    wqT = np.ascontiguousarray(np.asarray(Wq, np.float32).T).astype(bf)
    wkT = np.ascontiguousarray(np.asarray(Wk, np.float32).T).astype(bf)
    wvT = np.ascontiguousarray(np.asarray(Wv, np.float32).T).astype(bf)
    woT = np.ascontiguousarray(np.asarray(Wo, np.float32).T).astype(bf)
    bias = np.asarray(b_out, np.float32).reshape(C, 1)
    in_maps = []
    for c in range(NCORES):
        b, g = divmod(c, GROUP)
        i0 = g * SQ
        hsTb = hs[b].T  # [C, S]
        in_maps.append(
            {
                "hsT": np.ascontiguousarray(np.roll(hsTb, -i0, axis=1)).astype(bf),
                "res": np.ascontiguousarray(hsTb[:, i0 : i0 + SQ]) + bias,
                "wqT": wqT,
                "wkT": wkT,
                "wvT": wvT,
                "woT": woT,
            }
        )
    return in_maps


def assemble(results) -> np.ndarray:
    y = np.empty((B, S, C), np.float32)
    for c in range(NCORES):
        b, g = divmod(c, GROUP)
        i0 = g * SQ
        y[b, i0 : i0 + SQ, :] = np.asarray(results[c]["out"], np.float32).T
    return y


def kernel(**inputs) -> np.ndarray:
    from concourse.bass_utils import run_bass_kernel_spmd

    nc = get_nc()
    in_maps = make_in_maps(**inputs)
    res = run_bass_kernel_spmd(nc, in_maps, list(range(NCORES)))
    return assemble(res.results)


if __name__ == "__main__":
    import reference

    inputs = {k: np.asarray(v) for k, v in reference.setup_inputs().items()}
    got = kernel(**inputs)
    want = np.asarray(reference.reference(**inputs))
    err = np.linalg.norm(got - want) / np.linalg.norm(want)
    print("Relative error:", err)


# revision 13
# speedup vs baseline: 1.0504x; 1.0318x over previous
"""Multi-head attention (AttnProcessor2_0) on 8 TRN2 NeuronCores.

Problem: B=2, S=4096, C=640, H=10, Dh=64.
  q/k/v = hs @ W{q,k,v}.T ; per-head scores = q k^T / 8 ; softmax ;
  out = probs v ; y = out @ Wo.T + b_out + hs

Sharding (no collectives): core c -> batch b=c//4, query block g=c%4
(1024 queries).  Each core recomputes full K/V for its batch (head-dim
on partitions), computes its own S/4 x S attention block, output
projection, bias+residual.  Host passes hidden states TRANSPOSED and
ROLLED by the query offset so the same SPMD program works on every
core (softmax+PV are permutation-invariant along the key axis).

Device layout (everything feature-on-partition, token-on-free):
  kT [640, 4096] (5 chunks of 128 = 2 heads each)  "scoresT" = K Q^T
  qT [128, 1024] per head pair (rows 0:64 head even, 64:128 head odd)
  v  [4096, 650] (65-stride per head: 64 cols + ones col -> softmax
     denominators fall out of the PV matmul as PSUM row 64)
  QK: both heads of a pair run CONCURRENTLY as K=64 row-tiled matmuls
     (tile_position (0,0) and (64,0)) writing adjacent PSUM banks --
     2x the padded-contraction QK throughput of the old layout.
  probs: scoresT in PSUM -> ScalarE exp -> bf16 SBUF
  normalization: pv [65,512] copied to SBUF right after the PV
     accumulation stops (frees the PSUM bank), reciprocal of denom row,
     rank-1 PE outer product into a scratch PSUM bank to broadcast
     across partitions, DVE mult.
All matmuls bf16 (f32 PSUM accumulation).
"""

import sys

if "/opt/trn_rl_repo" not in sys.path:
    sys.path.insert(0, "/opt/trn_rl_repo")

from collections import deque
from contextlib import ExitStack

import ml_dtypes
import numpy as np

import concourse.bass as bass
import concourse.tile as tile
from concourse import mybir
from concourse.bass import ts

BF16 = mybir.dt.bfloat16
F32 = mybir.dt.float32

B, S, C = 2, 4096, 640
H, DH = 10, 64
NCORES = 8
GROUP = 4  # cores per batch element
SQ = S // GROUP  # 1024 queries per core
SCALE = 0.125  # 1/sqrt(64)
CCH = C // 128  # 5 feature chunks (2 heads each)
NJT = S // 512  # 8 key tiles for K proj
NJC = S // 128  # 32 key chunks for attention
NIT = SQ // 512  # 2 query tiles
VST = DH + 1  # 65: per-head stride in v tiles (ones col appended)

# Schraudolph exp offload: selected score chunks compute exp on the DVE
# as a bf16 bit-trick (one tensor_scalar: bits = round(s*A + B) viewed as
# bf16 gives 2^(s*log2e) with ~2% per-element jitter and ~zero mean; any
# constant bias cancels in the softmax ratio).  This moves work off the
# bottleneck ScalarE onto the (slack) DVE.  Set empty to disable.
SCHRAUD_A = SCALE * 128.0 / float(np.log(2.0))  # folds the 1/8 score scale
SCHRAUD_B = 127.0 * 128.0 - 7.45
OFFLOAD_JC = frozenset()


def build_nc() -> bass.Bass:
    nc = bass.Bass()
    hsT = nc.declare_dram_parameter("hsT", [C, S], BF16, isOutput=False)
    res = nc.declare_dram_parameter("res", [C, SQ], F32, isOutput=False)
    wqT = nc.declare_dram_parameter("wqT", [C, C], BF16, isOutput=False)
    wkT = nc.declare_dram_parameter("wkT", [C, C], BF16, isOutput=False)
    wvT = nc.declare_dram_parameter("wvT", [C, C], BF16, isOutput=False)
    woT = nc.declare_dram_parameter("woT", [C, C], BF16, isOutput=False)
    out = nc.declare_dram_parameter("out", [C, SQ], F32, isOutput=True)

    with ExitStack() as ctx:
        tc = ctx.enter_context(tile.TileContext(nc))
        # outer pool: tensors whose lifetime spans projections AND attention
        sb = ctx.enter_context(tc.tile_pool(name="sb", bufs=1))

        kT_sb = [sb.tile([128, S], BF16, tag=f"kT{i}", name=f"kT{i}") for i in range(CCH)]
        # head-pair q: rows 0:64 = even head, 64:128 = odd head.  The QK
        # matmuls are K=64 row-tiled (tile_position (0,0)/(64,0)) and run
        # concurrently in the PE array -- no zero padding needed.
        qT_sb = [sb.tile([128, SQ], BF16, tag=f"qT{i}", name=f"qT{i}") for i in range(CCH)]
        v_sb = [sb.tile([128, H * VST], BF16, tag=f"v{j}", name=f"v{j}") for j in range(NJC)]
        ones_sb = sb.tile([128, DH], BF16, tag="ones", name="ones")
        nc.vector.memset(ones_sb[:], 1.0)

        # ---------------- load + first projections ----------------
        # Only kT/qT for head pair 0 (keys 0:1024) are produced up front;
        # the remaining K/Q chunks, the whole V projection, and the it0
        # output projection are emitted as background work inside the
        # attention loop (2 spare PSUM banks, tag "pp").
        load = ctx.enter_context(tc.tile_pool(name="load", bufs=1))
        hsT_sb = []
        wq_sb, wk_sb, wv_sb = [], [], []
        for i in range(CCH):
            w = load.tile([128, C], BF16, tag=f"wk{i}", name=f"wk{i}")
            nc.sync.dma_start(w[:], wkT[ts(i, 128), :])
            wk_sb.append(w)
            t = load.tile([128, S], BF16, tag=f"hsT{i}", name=f"hsT{i}")
            nc.sync.dma_start(t[:, 0:SQ], hsT[ts(i, 128), 0:SQ])
            hsT_sb.append(t)
        for i in range(CCH):
            for name, lst, srcp in (("wq", wq_sb, wqT), ("wv", wv_sb, wvT)):
                w = load.tile([128, C], BF16, tag=f"{name}{i}", name=f"{name}{i}")
                nc.sync.dma_start(w[:], srcp[ts(i, 128), :])
                lst.append(w)
        # full Wo resident (800KB): kills the per-oproj weight DMAs and
        # zero-padding; with head-paired attn the contraction is all-real
        wo_sb = []
        for i in range(CCH):
            w = load.tile([128, C], BF16, tag=f"wo{i}", name=f"wo{i}")
            nc.sync.dma_start(w[:], woT[ts(i, 128), :])
            wo_sb.append(w)

        def emit_hsT_tail():
            # deferred until after the first exp so ScalarE's conservative
            # vector-clock waits don't cover this 4MB of DMA
            for blk in range(SQ, S, SQ):
                for i in range(CCH):
                    nc.sync.dma_start(
                        hsT_sb[i][:, blk : blk + SQ],
                        hsT[ts(i, 128), blk : blk + SQ],
                    )

        def proj_ops(w_sb, dst, dc, jt, pool):
            # one K/Q projection stripe as 6 micro-ops (5 MMs + cast) so
            # the background drain never inserts more than ~2 matmuls
            # between attention-stream matmuls (a whole 5-MM burst would
            # stall the exp pipeline ~600ns per burst)
            st = {}

            def mm(cc):
                def f():
                    if "ps" not in st:
                        st["ps"] = pool.tile([128, 512], F32, tag="pp",
                                             name="pp", bufs=2)
                    nc.tensor.matmul(
                        st["ps"][:],
                        w_sb[cc][:, ts(dc, 128)],
                        hsT_sb[cc][:, ts(jt, 512)],
                        start=(cc == 0),
                        stop=(cc == CCH - 1),
                    )
                return f

            def cast():
                nc.vector.tensor_copy(dst[:, ts(jt, 512)], st["ps"][:])

            return [mm(cc) for cc in range(CCH)] + [cast]

        def emit_kproj(dc, jt, pool):
            for f in proj_ops(wk_sb, kT_sb[dc], dc, jt, pool):
                f()

        def emit_qproj(dc, it, pool):
            for f in proj_ops(wq_sb, qT_sb[dc], dc, it, pool):
                f()

        def emit_vproj(jc, pool):
            vt = v_sb[jc]
            v3 = vt[:].rearrange("p (h x) -> p h x", x=VST)
            nc.vector.memset(v3[:, :, DH : DH + 1], 1.0)
            for d0, dn in ((0, 512), (512, 128)):
                ps = pool.tile([128, 512], F32, tag="pp", name="pp", bufs=2)
                for cc in range(CCH):
                    nc.tensor.matmul(
                        ps[:, 0:dn],
                        hsT_sb[cc][:, ts(jc, 128)],
                        wv_sb[cc][:, d0 : d0 + dn],
                        start=(cc == 0),
                        stop=(cc == CCH - 1),
                    )
                nc.vector.tensor_copy(
                    v3[:, d0 // DH : (d0 + dn) // DH, 0:DH],
                    ps[:, 0:dn].rearrange("p (h x) -> p h x", x=DH),
                )

        with tc.tile_pool(name="pp0", bufs=2, space="PSUM") as pp0:
            for jt in range(2):
                emit_kproj(0, jt, pp0)
            emit_qproj(0, 0, pp0)

        # ---------------- attention phase ----------------
        # attn2[hp]: head pair packed (rows 0:64 even head, 64:128 odd) --
        # the output projection contracts all 128 rows with no padding
        attn2_sb = [sb.tile([128, SQ], BF16, tag=f"attn{p}", name=f"attn{p}")
                    for p in range(CCH)]
        with tc.tile_pool(name="ap", bufs=1, space="PSUM") as ap, \
             tc.tile_pool(name="pt", bufs=4) as pt_pool, \
             tc.tile_pool(name="ob", bufs=3) as ob, \
             tc.tile_pool(name="scratch", bufs=3) as scratch:
            def norm_dve(hp, pv0, pv1, p_isl):
                # drain both pv accumulators into one packed tile (DVE
                # copies may shift partitions), denominators to rows 0/32
                # of a shared tile -> ONE reciprocal per head pair
                rawp = scratch.tile([128, 512], BF16, tag="raw", name="raw",
                                    bufs=2)
                nc.vector.tensor_copy(rawp[0:DH, :], pv0[0:DH, :])
                nc.vector.tensor_copy(rawp[DH:128, :], pv1[0:DH, :])
                rc = scratch.tile([33, 512], BF16, tag="rc", name="rc",
                                  bufs=2)
                dn = scratch.tile([33, 512], BF16, tag="dn", name="dn",
                                  bufs=2)
                nc.vector.tensor_copy(dn[0:1, :], pv0[DH : DH + 1, :])
                nc.vector.tensor_copy(dn[32:33, :], pv1[DH : DH + 1, :])
                with nc.allow_low_precision(reason="softmax recip bf16"):
                    nc.vector.reciprocal(rc[:], dn[:])
                return (hp, p_isl, rc, rawp)

            def norm_pe(state, anchor, pool):
                # two concurrent rank-1 PE broadcasts of the reciprocals
                # (row/col tiles (0,0) and (32,64)), pinned behind the
                # anchor QK so the slow DVE reciprocal is hidden
                hp, p_isl, rc, rawp = state
                ps = pool.tile([128, 512], F32, tag="pp", name="pp", bufs=2)
                r_mm = nc.tensor.matmul(
                    ps[0:DH, :],
                    ones_sb[0:1, 0:DH],
                    rc[0:1, :],
                    start=True,
                    stop=True,
                )
                nc.tensor.matmul(
                    ps[DH:128, :],
                    ones_sb[32:33, 0:DH],
                    rc[32:33, :],
                    start=True,
                    stop=True,
                )
                if anchor is not None:
                    tile.add_dep_helper(
                        r_mm.ins, anchor.ins, sync=False,
                        reason="norm R after anchor QK (hide recip latency)",
                    )
                nc.vector.tensor_mul(
                    attn2_sb[hp][:, p_isl], rawp[:], ps[:]
                )

            def oproj_ops(ec, it):
                st = {}

                def mm(r):
                    def f():
                        if "ps" not in st:
                            st["ps"] = ap.tile([128, 512], F32, tag="pp",
                                               name="pp", bufs=2)
                            rt = ob.tile([128, 512], F32, tag="rt",
                                         name="rt", bufs=2)
                            nc.sync.dma_start(
                                rt[:], res[ts(ec, 128), ts(it, 512)]
                            )
                            st["rt"] = rt
                        nc.tensor.matmul(
                            st["ps"][:],
                            wo_sb[r][:, ts(ec, 128)],
                            attn2_sb[r][:, ts(it, 512)],
                            start=(r == 0),
                            stop=(r == CCH - 1),
                        )
                    return f

                def fin():
                    ot = ob.tile([128, 512], F32, tag="ot", name="ot", bufs=2)
                    nc.vector.tensor_add(ot[:], st["ps"][:], st["rt"][:])
                    nc.sync.dma_start(out[ts(ec, 128), ts(it, 512)], ot[:])

                return [mm(r) for r in range(CCH)] + [fin]

            def emit_oproj(ec, it):
                for f in oproj_ops(ec, it):
                    f()

            # Background work flows through a micro-op queue drained at
            # most 2 ops per jc step, so no more than ~2 weight matmuls
            # ever sit between attention-stream matmuls on the PE (a
            # whole 5-MM projection burst stalls the exp pipeline).
            # Deadlines: a window's own kT stripes jt2..7 (first used at
            # jc8/12/../28) are enqueued at window start and finish by
            # ~jc17; the next window's kT jt0/jt1 + qT slice drain by the
            # window's end.  it1 windows carry the it0 output projections
            # and lazily-deferred it1 q-projections.
            bgq = deque()

            pending = []
            for it in range(NIT):
                isl = ts(it, 512)
                for hp in range(CCH):
                    h0, h1 = 2 * hp, 2 * hp + 1
                    if it == 0:
                        for jt in range(2, NJT):
                            bgq.extend(proj_ops(wk_sb, kT_sb[hp], hp, jt, ap))
                        if hp < CCH - 1:
                            for jt in range(2):
                                bgq.extend(
                                    proj_ops(wk_sb, kT_sb[hp + 1], hp + 1, jt, ap)
                                )
                            bgq.extend(proj_ops(wq_sb, qT_sb[hp + 1], hp + 1, 0, ap))
                        else:
                            bgq.extend(proj_ops(wq_sb, qT_sb[0], 0, 1, ap))
                    else:
                        if hp == 0:
                            bgq.extend(proj_ops(wq_sb, qT_sb[1], 1, 1, ap))
                        elif hp == 1:
                            bgq.extend(oproj_ops(0, 0))
                            bgq.extend(oproj_ops(1, 0))
                            bgq.extend(proj_ops(wq_sb, qT_sb[2], 2, 1, ap))
                        elif hp == 2:
                            bgq.extend(oproj_ops(2, 0))
                            bgq.extend(proj_ops(wq_sb, qT_sb[3], 3, 1, ap))
                        elif hp == 3:
                            bgq.extend(oproj_ops(3, 0))
                            bgq.extend(proj_ops(wq_sb, qT_sb[4], 4, 1, ap))
                        else:
                            bgq.extend(oproj_ops(4, 0))
                    vtodo = {}
                    if it == 0 and hp == 0:
                        # V chunks 0..7 front-loaded (keys 0:1024 resident
                        # before the hsT tail lands), then one chunk per
                        # step four steps ahead of its PV use
                        for jc in range(4):
                            vtodo[jc] = [2 * jc, 2 * jc + 1]
                        for jc in range(4, 28):
                            vtodo[jc] = [jc + 4]
                    pv0 = ap.tile([DH + 1, 512], F32, tag="pv0", bufs=1,
                                  name="pv0")
                    pv1 = ap.tile([DH + 1, 512], F32, tag="pv1", bufs=1,
                                  name="pv1")
                    for jc in range(NJC):
                        sc = ap.tile([128, 1024], F32, tag="sc", bufs=2,
                                     name="sc")
                        qk0 = nc.tensor.matmul(
                            sc[:, 0:512],
                            kT_sb[hp][0:DH, ts(jc, 128)],
                            qT_sb[hp][0:DH, isl],
                            start=True,
                            stop=True,
                        )
                        nc.tensor.matmul(
                            sc[:, 512:1024],
                            kT_sb[hp][DH:128, ts(jc, 128)],
                            qT_sb[hp][DH:128, isl],
                            start=True,
                            stop=True,
                        )
                        pt = pt_pool.tile([128, 1024], BF16,
                                          tag="pt", name="pt")
                        if (it, hp) != (0, 0) and jc in OFFLOAD_JC:
                            nc.vector.tensor_scalar(
                                out=pt[:].bitcast(mybir.dt.int16),
                                in0=sc[:],
                                scalar1=SCHRAUD_A,
                                scalar2=SCHRAUD_B,
                                op0=mybir.AluOpType.mult,
                                op1=mybir.AluOpType.add,
                            )
                        else:
                            nc.scalar.activation(
                                pt[:], sc[:],
                                mybir.ActivationFunctionType.Exp,
                                bias=0.0, scale=SCALE,
                            )
                        if it == 0 and hp == 0 and jc == 0:
                            emit_hsT_tail()
                        if pending and jc == 5:
                            norm_pe(pending.pop(0), qk0, ap)
                        for j in vtodo.get(jc, ()):
                            emit_vproj(j, ap)
                        for _ in range(2):
                            if bgq:
                                bgq.popleft()()
                        nc.tensor.matmul(
                            pv0[:],
                            v_sb[jc][:, h0 * VST : (h0 + 1) * VST],
                            pt[:, 0:512],
                            start=(jc == 0),
                            stop=(jc == NJC - 1),
                        )
                        nc.tensor.matmul(
                            pv1[:],
                            v_sb[jc][:, h1 * VST : (h1 + 1) * VST],
                            pt[:, 512:1024],
                            start=(jc == 0),
                            stop=(jc == NJC - 1),
                        )
                    pending.append(norm_dve(hp, pv0, pv1, isl))
            while bgq:
                bgq.popleft()()
            for st in pending:
                norm_pe(st, None, ap)
            for ec in range(CCH):
                emit_oproj(ec, 1)

    _spill_matmul_waits(nc)
    return nc


# walrus embedded-sync-wait capacity per BIR opcode.  Matmult holds a
# single wait; excess waits hoist onto the paired Ldweights (in-order
# issue on PE makes that equivalent).  Other compute ops spill onto
# EventSemaphore carrier instructions inserted just before them on the
# same engine.  DMACopy / Drain / EventSemaphore handle many waits
# natively (bacc emits such itself) and are left alone.
_WAIT_CAPS = {
    "InstMatmult": 1,
    "InstLdweights": 1,
    "InstActivation": 1,
    "InstReciprocal": 1,
    "InstTensorTensor": 1,
    "InstTensorCopy": 1,
    "InstTensorScalarPtr": 1,
    "InstTensorReduce": 1,
    "InstMemset": 1,
    "InstDMACopy": 1,
    "InstDrain": 1,
    "InstCustomDveAnt": 1,
}
_ES_CAP = 2  # waits per EventSemaphore carrier (walrus: <=2 waits, <=1 update)


def _spill_matmul_waits(nc: bass.Bass) -> None:
    spill_id = [0]

    def carriers(excess, engine):
        out = []
        for i in range(0, len(excess), _ES_CAP):
            es = mybir.InstEventSemaphore(
                name=f"wait-spill-{spill_id[0]}", ins=[], outs=[]
            )
            spill_id[0] += 1
            es.engine = engine
            es.sync_info = mybir.SyncInfo(
                on_wait=excess[i : i + _ES_CAP], on_update=[]
            )
            out.append(es)
        return out

    for f in nc.m.functions:
        for blk in f.blocks:
            insts = blk.instructions
            i = 0
            while i < len(insts):
                inst = insts[i]
                tn = type(inst).__name__
                cap = _WAIT_CAPS.get(tn)
                si = inst.sync_info
                if cap is None or si is None or len(si.on_wait) <= cap:
                    i += 1
                    continue
                w = list(si.on_wait)
                if tn == "InstMatmult" and cap == 1:
                    # Keep the latest-satisfied dependency (the ACT-produced
                    # operand, e.g. probs from exp) embedded on the matmul and
                    # hoist early ones onto the Ldweights: a wait on the LDW
                    # blocks its background prefetch and serializes ~50ns of
                    # weight-load into every PV matmul.
                    acts = [x for x in w if "Activation" in (x.ant_name or "")]
                    if acts:
                        keep = [acts[-1]]
                        excess = [x for x in w if x is not acts[-1]]
                    else:
                        keep, excess = w[-cap:], w[:-cap]
                else:
                    keep, excess = w[-cap:], w[:-cap]
                prev = insts[i - 1] if i > 0 else None
                if (
                    tn == "InstMatmult"
                    and prev is not None
                    and type(prev).__name__ == "InstLdweights"
                    and len(((prev.sync_info and prev.sync_info.on_wait) or []))
                    + len(excess) <= 1
                ):
                    psi = prev.sync_info
                    pw = list(psi.on_wait) if psi is not None else []
                    pu = list(psi.on_update) if psi is not None else []
                    prev.sync_info = mybir.SyncInfo(on_wait=pw + excess, on_update=pu)
                else:
                    new = carriers(excess, inst.engine)
                    insts[i:i] = new
                    i += len(new)
                inst.sync_info = mybir.SyncInfo(
                    on_wait=keep, on_update=list(si.on_update)
                )
                i += 1


_CACHED_NC = None


def get_nc() -> bass.Bass:
    global _CACHED_NC
    if _CACHED_NC is None:
        _CACHED_NC = build_nc()
    return _CACHED_NC


def make_in_maps(hidden_states, Wq, Wk, Wv, Wo, b_out):
    hs = np.asarray(hidden_states, dtype=np.float32)
    bf = ml_dtypes.bfloat16
    wqT = np.ascontiguousarray(np.asarray(Wq, np.float32).T).astype(bf)
    wkT = np.ascontiguousarray(np.asarray(Wk, np.float32).T).astype(bf)
    wvT = np.ascontiguousarray(np.asarray(Wv, np.float32).T).astype(bf)
    woT = np.ascontiguousarray(np.asarray(Wo, np.float32).T).astype(bf)
    bias = np.asarray(b_out, np.float32).reshape(C, 1)
    in_maps = []
    for c in range(NCORES):
        b, g = divmod(c, GROUP)
        i0 = g * SQ
        hsTb = hs[b].T  # [C, S]
        in_maps.append(
            {
                "hsT": np.ascontiguousarray(np.roll(hsTb, -i0, axis=1)).astype(bf),
                "res": np.ascontiguousarray(hsTb[:, i0 : i0 + SQ]) + bias,
                "wqT": wqT,
                "wkT": wkT,
                "wvT": wvT,
                "woT": woT,
            }
        )
    return in_maps


def assemble(results) -> np.ndarray:
    y = np.empty((B, S, C), np.float32)
    for c in range(NCORES):
        b, g = divmod(c, GROUP)
        i0 = g * SQ
        y[b, i0 : i0 + SQ, :] = np.asarray(results[c]["out"], np.float32).T
    return y


def kernel(**inputs) -> np.ndarray:
    from concourse.bass_utils import run_bass_kernel_spmd

    nc = get_nc()
    in_maps = make_in_maps(**inputs)
    res = run_bass_kernel_spmd(nc, in_maps, list(range(NCORES)))
    return assemble(res.results)


if __name__ == "__main__":
    import reference

    inputs = {k: np.asarray(v) for k, v in reference.setup_inputs().items()}
    got = kernel(**inputs)
    want = np.asarray(reference.reference(**inputs))
    err = np.linalg.norm(got - want) / np.linalg.norm(want)
    print("Relative error:", err)


# revision 25
# speedup vs baseline: 1.0736x; 1.0221x over previous
"""Multi-head attention (AttnProcessor2_0) on 8 TRN2 NeuronCores.

Problem: B=2, S=4096, C=640, H=10, Dh=64.
  q/k/v = hs @ W{q,k,v}.T ; per-head scores = q k^T / 8 ; softmax ;
  out = probs v ; y = out @ Wo.T + b_out + hs

Sharding (no collectives): core c -> batch b=c//4, query block g=c%4
(1024 queries).  Each core recomputes full K/V for its batch (head-dim
on partitions), computes its own S/4 x S attention block, output
projection, bias+residual.  Host passes hidden states TRANSPOSED and
ROLLED by the query offset so the same SPMD program works on every
core (softmax+PV are permutation-invariant along the key axis).

Device layout (everything feature-on-partition, token-on-free):
  kT [640, 4096] (5 chunks of 128 = 2 heads each)  "scoresT" = K Q^T
  qT [128, 1024] per head pair (rows 0:64 head even, 64:128 head odd)
  v  [4096, 650] (65-stride per head: 64 cols + ones col -> softmax
     denominators fall out of the PV matmul as PSUM row 64)
  QK: both heads of a pair run CONCURRENTLY as K=64 row-tiled matmuls
     (tile_position (0,0) and (64,0)) writing adjacent PSUM banks --
     2x the padded-contraction QK throughput of the old layout.
  probs: scoresT in PSUM -> ScalarE exp -> bf16 SBUF
  normalization: pv [65,512] copied to SBUF right after the PV
     accumulation stops (frees the PSUM bank), reciprocal of denom row,
     rank-1 PE outer product into a scratch PSUM bank to broadcast
     across partitions, DVE mult.
All matmuls bf16 (f32 PSUM accumulation).
"""

import sys

if "/opt/trn_rl_repo" not in sys.path:
    sys.path.insert(0, "/opt/trn_rl_repo")

from collections import deque
from contextlib import ExitStack

import ml_dtypes
import numpy as np

import concourse.bass as bass
import concourse.tile as tile
from concourse import mybir
from concourse.bass import ts

BF16 = mybir.dt.bfloat16
F32 = mybir.dt.float32

B, S, C = 2, 4096, 640
H, DH = 10, 64
NCORES = 8
GROUP = 4  # cores per batch element
SQ = S // GROUP  # 1024 queries per core
SCALE = 0.125  # 1/sqrt(64)
CCH = C // 128  # 5 feature chunks (2 heads each)
NJT = S // 512  # 8 key tiles for K proj
NJC = S // 128  # 32 key chunks for attention
NIT = SQ // 512  # 2 query tiles
VST = DH + 1  # 65: per-head stride in v tiles (ones col appended)

# Schraudolph exp offload: selected score chunks compute exp on the DVE
# as a bf16 bit-trick (one tensor_scalar: bits = round(s*A + B) viewed as
# bf16 gives 2^(s*log2e) with ~2% per-element jitter and ~zero mean; any
# constant bias cancels in the softmax ratio).  This moves work off the
# bottleneck ScalarE onto the (slack) DVE.  Set empty to disable.
SCHRAUD_A = SCALE * 128.0 / float(np.log(2.0))  # folds the 1/8 score scale
SCHRAUD_B = 127.0 * 128.0 - 7.45
OFFLOAD_JC = frozenset()


def build_nc() -> bass.Bass:
    nc = bass.Bass()
    hsT = nc.declare_dram_parameter("hsT", [C, S], BF16, isOutput=False)
    res = nc.declare_dram_parameter("res", [C, SQ], F32, isOutput=False)
    wqT = nc.declare_dram_parameter("wqT", [C, C], BF16, isOutput=False)
    wkT = nc.declare_dram_parameter("wkT", [C, C], BF16, isOutput=False)
    wvT = nc.declare_dram_parameter("wvT", [C, C], BF16, isOutput=False)
    woT = nc.declare_dram_parameter("woT", [C, C], BF16, isOutput=False)
    out = nc.declare_dram_parameter("out", [C, SQ], F32, isOutput=True)

    with ExitStack() as ctx:
        tc = ctx.enter_context(tile.TileContext(nc))
        # outer pool: tensors whose lifetime spans projections AND attention
        sb = ctx.enter_context(tc.tile_pool(name="sb", bufs=1))

        kT_sb = [sb.tile([128, S], BF16, tag=f"kT{i}", name=f"kT{i}") for i in range(CCH)]
        # head-pair q: rows 0:64 = even head, 64:128 = odd head.  The QK
        # matmuls are K=64 row-tiled (tile_position (0,0)/(64,0)) and run
        # concurrently in the PE array -- no zero padding needed.
        qT_sb = [sb.tile([128, SQ], BF16, tag=f"qT{i}", name=f"qT{i}") for i in range(CCH)]
        v_sb = [sb.tile([128, H * VST], BF16, tag=f"v{j}", name=f"v{j}") for j in range(NJC)]
        ones_sb = sb.tile([128, DH], BF16, tag="ones", name="ones")
        nc.vector.memset(ones_sb[:], 1.0)

        # prefetch the exp table set while DMAs stream (the pseudo
        # ACT_TABLE_LOAD walrus inserts before the first real exp would
        # otherwise land on the critical path, ~1.3us)
        warm = sb.tile([1, 16], F32, tag="warm", name="warm")
        nc.vector.memset(warm[:], 0.0)
        nc.scalar.activation(warm[:], warm[:],
                             mybir.ActivationFunctionType.Exp,
                             bias=0.0, scale=0.0)

        # ---------------- load + first projections ----------------
        # Each input tensor is ONE wide SBUF tile filled by ONE DMA (the
        # Sync engine issues triggers at ~600ns each -- 20 small DMAs
        # serialized the old startup).  Chunk cc of a tensor lives at
        # free-offset cc*width; h3/wk3/... are [128, chunk, width] views.
        load = ctx.enter_context(tc.tile_pool(name="load", bufs=1))
        hsT_big = load.tile([128, CCH * S], BF16, tag="hsT", name="hsT")
        h3 = hsT_big[:].rearrange("p (f s) -> p f s", s=S)
        wk3 = load.tile([128, CCH * C], BF16, tag="wk", name="wk")[:] \
            .rearrange("p (f c) -> p f c", c=C)
        wq3 = load.tile([128, CCH * C], BF16, tag="wq", name="wq")[:] \
            .rearrange("p (f c) -> p f c", c=C)
        wv3 = load.tile([128, CCH * C], BF16, tag="wv", name="wv")[:] \
            .rearrange("p (f c) -> p f c", c=C)
        # full Wo resident (800KB): kills the per-oproj weight DMAs and
        # zero-padding; with head-paired attn the contraction is all-real
        wo3 = load.tile([128, CCH * C], BF16, tag="wo", name="wo")[:] \
            .rearrange("p (f c) -> p f c", c=C)
        nc.sync.dma_start(wk3, wkT[:, :].rearrange("(f p) c -> p f c", p=128))
        nc.sync.dma_start(
            h3[:, :, 0:SQ],
            hsT[:, 0:SQ].rearrange("(f p) s -> p f s", p=128),
        )
        nc.sync.dma_start(wq3, wqT[:, :].rearrange("(f p) c -> p f c", p=128))
        nc.sync.dma_start(wv3, wvT[:, :].rearrange("(f p) c -> p f c", p=128))
        nc.sync.dma_start(wo3, woT[:, :].rearrange("(f p) c -> p f c", p=128))

        def emit_hsT_tail():
            # deferred until after the first exp so ScalarE's conservative
            # vector-clock waits don't cover this 4MB of DMA
            for blk in range(SQ, S, SQ):
                nc.sync.dma_start(
                    h3[:, :, blk : blk + SQ],
                    hsT[:, blk : blk + SQ].rearrange("(f p) s -> p f s", p=128),
                )

        def proj_ops(w3, dst, dc, jt, pool):
            # one K/Q projection stripe as 6 micro-ops (5 MMs + cast) so
            # the background drain never inserts more than ~2 matmuls
            # between attention-stream matmuls (a whole 5-MM burst would
            # stall the exp pipeline ~600ns per burst)
            st = {}

            def mm(cc):
                def f():
                    if "ps" not in st:
                        st["ps"] = pool.tile([128, 512], F32, tag="pp",
                                             name="pp", bufs=2)
                    nc.tensor.matmul(
                        st["ps"][:],
                        w3[:, cc, ts(dc, 128)],
                        h3[:, cc, ts(jt, 512)],
                        start=(cc == 0),
                        stop=(cc == CCH - 1),
                    )
                return f

            def cast():
                nc.vector.tensor_copy(dst[:, ts(jt, 512)], st["ps"][:])

            return [mm(cc) for cc in range(CCH)] + [cast]

        def emit_kproj(dc, jt, pool):
            for f in proj_ops(wk3, kT_sb[dc], dc, jt, pool):
                f()

        def emit_qproj(dc, it, pool):
            for f in proj_ops(wq3, qT_sb[dc], dc, it, pool):
                f()

        def emit_vproj(jc, pool):
            vt = v_sb[jc]
            v3 = vt[:].rearrange("p (h x) -> p h x", x=VST)
            for d0, dn in ((0, 512), (512, 128)):
                ps = pool.tile([128, 512], F32, tag="pp", name="pp", bufs=2)
                for cc in range(CCH):
                    nc.tensor.matmul(
                        ps[:, 0:dn],
                        h3[:, cc, ts(jc, 128)],
                        wv3[:, cc, d0 : d0 + dn],
                        start=(cc == 0),
                        stop=(cc == CCH - 1),
                    )
                nc.vector.tensor_copy(
                    v3[:, d0 // DH : (d0 + dn) // DH, 0:DH],
                    ps[:, 0:dn].rearrange("p (h x) -> p h x", x=DH),
                )

        # ones columns of all v tiles set once up front (DVE is idle
        # during the DMA-bound startup; doing this inside window 0 cost
        # ~0.7us of DVE per chunk right where the PE is most oversubscribed)
        for jc in range(NJC):
            v3c = v_sb[jc][:].rearrange("p (h x) -> p h x", x=VST)
            nc.vector.memset(v3c[:, :, DH : DH + 1], 1.0)

        with tc.tile_pool(name="pp0", bufs=2, space="PSUM") as pp0:
            for jt in range(2):
                emit_kproj(0, jt, pp0)
            emit_qproj(0, 0, pp0)

        # ---------------- attention phase ----------------
        # attn2[hp]: head pair packed (rows 0:64 even head, 64:128 odd) --
        # the output projection contracts all 128 rows with no padding
        attn2_sb = [sb.tile([128, SQ], BF16, tag=f"attn{p}", name=f"attn{p}")
                    for p in range(CCH)]
        with tc.tile_pool(name="ap", bufs=1, space="PSUM") as ap, \
             tc.tile_pool(name="pt", bufs=6) as pt_pool, \
             tc.tile_pool(name="ob", bufs=3) as ob, \
             tc.tile_pool(name="scratch", bufs=3) as scratch:
            def norm_dve(hp, pv0, pv1, p_isl, tail=False):
                # drain both pv accumulators into one packed tile (DVE
                # copies may shift partitions), denominators to rows 0/32
                # of a shared tile -> ONE reciprocal per head pair
                rawp = scratch.tile([128, 512], BF16, tag="raw", name="raw",
                                    bufs=2)
                nc.vector.tensor_copy(rawp[0:DH, :], pv0[0:DH, :])
                nc.vector.tensor_copy(rawp[DH:128, :], pv1[0:DH, :])
                rc = scratch.tile([33, 512], BF16, tag="rc", name="rc",
                                  bufs=2)
                dn = scratch.tile([33, 512], BF16, tag="dn", name="dn",
                                  bufs=2)
                nc.vector.tensor_copy(dn[0:1, :], pv0[DH : DH + 1, :])
                nc.vector.tensor_copy(dn[32:33, :], pv1[DH : DH + 1, :])
                with nc.allow_low_precision(reason="softmax recip bf16"):
                    if tail:
                        # tail: ScalarE is idle and the DVE reciprocal
                        # (3.3us, 8 cyc/elem iterative divide) would gate
                        # the final output projection; 1/x = exp(-ln(x))
                        # costs 2 short ACTs instead
                        lg = scratch.tile([33, 512], F32, tag="lg",
                                          name="lg", bufs=2)
                        nc.scalar.activation(
                            lg[:], dn[:], mybir.ActivationFunctionType.Ln,
                            bias=0.0, scale=1.0,
                        )
                        nc.scalar.activation(
                            rc[:], lg[:], mybir.ActivationFunctionType.Exp,
                            bias=0.0, scale=-1.0,
                        )
                    else:
                        nc.vector.reciprocal(rc[:], dn[:])
                return (hp, p_isl, rc, rawp)

            def norm_pe(state, anchor, pool):
                # two concurrent rank-1 PE broadcasts of the reciprocals
                # (row/col tiles (0,0) and (32,64)), pinned behind the
                # anchor QK so the slow DVE reciprocal is hidden
                hp, p_isl, rc, rawp = state
                ps = pool.tile([128, 512], F32, tag="pp", name="pp", bufs=2)
                r_mm = nc.tensor.matmul(
                    ps[0:DH, :],
                    ones_sb[0:1, 0:DH],
                    rc[0:1, :],
                    start=True,
                    stop=True,
                )
                nc.tensor.matmul(
                    ps[DH:128, :],
                    ones_sb[32:33, 0:DH],
                    rc[32:33, :],
                    start=True,
                    stop=True,
                )
                if anchor is not None:
                    tile.add_dep_helper(
                        r_mm.ins, anchor.ins, sync=False,
                        reason="norm R after anchor QK (hide recip latency)",
                    )
                nc.vector.tensor_mul(
                    attn2_sb[hp][:, p_isl], rawp[:], ps[:]
                )

            def oproj_ops(ec, it):
                st = {}

                def mm(r):
                    def f():
                        if "ps" not in st:
                            st["ps"] = ap.tile([128, 512], F32, tag="pp",
                                               name="pp", bufs=2)
                            rt = ob.tile([128, 512], F32, tag="rt",
                                         name="rt", bufs=2)
                            nc.sync.dma_start(
                                rt[:], res[ts(ec, 128), ts(it, 512)]
                            )
                            st["rt"] = rt
                        nc.tensor.matmul(
                            st["ps"][:],
                            wo3[:, r, ts(ec, 128)],
                            attn2_sb[r][:, ts(it, 512)],
                            start=(r == 0),
                            stop=(r == CCH - 1),
                        )
                    return f

                def fin():
                    ot = ob.tile([128, 512], F32, tag="ot", name="ot", bufs=2)
                    nc.vector.tensor_add(ot[:], st["ps"][:], st["rt"][:])
                    nc.sync.dma_start(out[ts(ec, 128), ts(it, 512)], ot[:])

                return [mm(r) for r in range(CCH)] + [fin]

            def emit_oproj(ec, it):
                for f in oproj_ops(ec, it):
                    f()

            # Background work flows through a micro-op queue drained at
            # most 2 ops per jc step, so no more than ~2 weight matmuls
            # ever sit between attention-stream matmuls on the PE (a
            # whole 5-MM projection burst stalls the exp pipeline).
            # Deadlines: a window's own kT stripes jt2..7 (first used at
            # jc8/12/../28) are enqueued at window start and finish by
            # ~jc17; the next window's kT jt0/jt1 + qT slice drain by the
            # window's end.  it1 windows carry the it0 output projections
            # and lazily-deferred it1 q-projections.
            bgq = deque()

            pending = []
            for it in range(NIT):
                isl = ts(it, 512)
                for hp in range(CCH):
                    h0, h1 = 2 * hp, 2 * hp + 1
                    if it == 0:
                        for jt in range(2, NJT):
                            bgq.extend(proj_ops(wk3, kT_sb[hp], hp, jt, ap))
                        if hp < CCH - 1:
                            for jt in range(2):
                                bgq.extend(
                                    proj_ops(wk3, kT_sb[hp + 1], hp + 1, jt, ap)
                                )
                            bgq.extend(proj_ops(wq3, qT_sb[hp + 1], hp + 1, 0, ap))
                        else:
                            bgq.extend(proj_ops(wq3, qT_sb[0], 0, 1, ap))
                    else:
                        if hp == 0:
                            bgq.extend(proj_ops(wq3, qT_sb[1], 1, 1, ap))
                        elif hp == 1:
                            bgq.extend(oproj_ops(0, 0))
                            bgq.extend(oproj_ops(1, 0))
                            bgq.extend(proj_ops(wq3, qT_sb[2], 2, 1, ap))
                        elif hp == 2:
                            bgq.extend(oproj_ops(2, 0))
                            bgq.extend(proj_ops(wq3, qT_sb[3], 3, 1, ap))
                        elif hp == 3:
                            bgq.extend(oproj_ops(3, 0))
                            bgq.extend(proj_ops(wq3, qT_sb[4], 4, 1, ap))
                        else:
                            bgq.extend(oproj_ops(4, 0))
                    vtodo = {}
                    if it == 0 and hp == 0:
                        # V chunks 0..7 front-loaded (keys 0:1024 resident
                        # before the hsT tail lands), then one chunk per
                        # step four steps ahead of its PV use
                        for jc in range(4):
                            vtodo[jc] = [2 * jc, 2 * jc + 1]
                        for jc in range(4, 28):
                            vtodo[jc] = [jc + 4]
                    pv0 = ap.tile([DH + 1, 512], F32, tag="pv0", bufs=1,
                                  name="pv0")
                    pv1 = ap.tile([DH + 1, 512], F32, tag="pv1", bufs=1,
                                  name="pv1")
                    for jc in range(NJC):
                        sc = ap.tile([128, 1024], F32, tag="sc", bufs=2,
                                     name="sc")
                        qk0 = nc.tensor.matmul(
                            sc[:, 0:512],
                            kT_sb[hp][0:DH, ts(jc, 128)],
                            qT_sb[hp][0:DH, isl],
                            start=True,
                            stop=True,
                        )
                        nc.tensor.matmul(
                            sc[:, 512:1024],
                            kT_sb[hp][DH:128, ts(jc, 128)],
                            qT_sb[hp][DH:128, isl],
                            start=True,
                            stop=True,
                        )
                        pt = pt_pool.tile([128, 1024], BF16,
                                          tag="pt", name="pt")
                        if (it, hp) != (0, 0) and jc in OFFLOAD_JC:
                            nc.vector.tensor_scalar(
                                out=pt[:].bitcast(mybir.dt.int16),
                                in0=sc[:],
                                scalar1=SCHRAUD_A,
                                scalar2=SCHRAUD_B,
                                op0=mybir.AluOpType.mult,
                                op1=mybir.AluOpType.add,
                            )
                        else:
                            nc.scalar.activation(
                                pt[:], sc[:],
                                mybir.ActivationFunctionType.Exp,
                                bias=0.0, scale=SCALE,
                            )
                        if it == 0 and hp == 0 and jc == 0:
                            emit_hsT_tail()
                        if pending and jc == 5:
                            norm_pe(pending.pop(0), qk0, ap)
                        for j in vtodo.get(jc, ()):
                            emit_vproj(j, ap)
                        for _ in range(2):
                            if bgq:
                                bgq.popleft()()
                        nc.tensor.matmul(
                            pv0[:],
                            v_sb[jc][:, h0 * VST : (h0 + 1) * VST],
                            pt[:, 0:512],
                            start=(jc == 0),
                            stop=(jc == NJC - 1),
                        )
                        nc.tensor.matmul(
                            pv1[:],
                            v_sb[jc][:, h1 * VST : (h1 + 1) * VST],
                            pt[:, 512:1024],
                            start=(jc == 0),
                            stop=(jc == NJC - 1),
                        )
                    pending.append(norm_dve(hp, pv0, pv1, isl,
                                            tail=(it == 1 and hp == CCH - 1)))
            while bgq:
                bgq.popleft()()
            # tail: the final pair's attn2[4] gates only the r=4 matmul of
            # each output projection -- accumulate r=0..3 for one ec (one
            # pp PSUM slot; the other must stay free for the norm's R)
            # underneath the reciprocal, then finish
            tail_ops = [oproj_ops(ec, 1) for ec in range(CCH)]
            for f in tail_ops[0][0:4]:
                f()
            for st in pending:
                norm_pe(st, None, ap)
            for ec in range(CCH):
                for f in tail_ops[ec][4:] if ec < 1 else tail_ops[ec]:
                    f()

    _spill_matmul_waits(nc)
    return nc


# walrus embedded-sync-wait capacity per BIR opcode.  Matmult holds a
# single wait; excess waits hoist onto the paired Ldweights (in-order
# issue on PE makes that equivalent).  Other compute ops spill onto
# EventSemaphore carrier instructions inserted just before them on the
# same engine.  DMACopy / Drain / EventSemaphore handle many waits
# natively (bacc emits such itself) and are left alone.
_WAIT_CAPS = {
    "InstMatmult": 1,
    "InstLdweights": 1,
    "InstActivation": 1,
    "InstReciprocal": 1,
    "InstTensorTensor": 1,
    "InstTensorCopy": 1,
    "InstTensorScalarPtr": 1,
    "InstTensorReduce": 1,
    "InstMemset": 1,
    "InstDMACopy": 1,
    "InstDrain": 1,
    "InstCustomDveAnt": 1,
}
_ES_CAP = 2  # waits per EventSemaphore carrier (walrus: <=2 waits, <=1 update)


def _spill_matmul_waits(nc: bass.Bass) -> None:
    spill_id = [0]

    def carriers(excess, engine):
        out = []
        for i in range(0, len(excess), _ES_CAP):
            es = mybir.InstEventSemaphore(
                name=f"wait-spill-{spill_id[0]}", ins=[], outs=[]
            )
            spill_id[0] += 1
            es.engine = engine
            es.sync_info = mybir.SyncInfo(
                on_wait=excess[i : i + _ES_CAP], on_update=[]
            )
            out.append(es)
        return out

    for f in nc.m.functions:
        for blk in f.blocks:
            insts = blk.instructions
            i = 0
            while i < len(insts):
                inst = insts[i]
                tn = type(inst).__name__
                cap = _WAIT_CAPS.get(tn)
                si = inst.sync_info
                if cap is None or si is None or len(si.on_wait) <= cap:
                    i += 1
                    continue
                w = list(si.on_wait)
                if tn == "InstMatmult" and cap == 1:
                    # Keep the latest-satisfied dependency (the ACT-produced
                    # operand, e.g. probs from exp) embedded on the matmul and
                    # hoist early ones onto the Ldweights: a wait on the LDW
                    # blocks its background prefetch and serializes ~50ns of
                    # weight-load into every PV matmul.
                    acts = [x for x in w if "Activation" in (x.ant_name or "")]
                    if acts:
                        keep = [acts[-1]]
                        excess = [x for x in w if x is not acts[-1]]
                    else:
                        keep, excess = w[-cap:], w[:-cap]
                else:
                    keep, excess = w[-cap:], w[:-cap]
                prev = insts[i - 1] if i > 0 else None
                if (
                    tn == "InstMatmult"
                    and prev is not None
                    and type(prev).__name__ == "InstLdweights"
                    and len(((prev.sync_info and prev.sync_info.on_wait) or []))
                    + len(excess) <= 1
                ):
                    psi = prev.sync_info
                    pw = list(psi.on_wait) if psi is not None else []
                    pu = list(psi.on_update) if psi is not None else []
                    prev.sync_info = mybir.SyncInfo(on_wait=pw + excess, on_update=pu)
                else:
                    new = carriers(excess, inst.engine)
                    insts[i:i] = new
                    i += len(new)
                inst.sync_info = mybir.SyncInfo(
                    on_wait=keep, on_update=list(si.on_update)
                )
                i += 1


_CACHED_NC = None


def get_nc() -> bass.Bass:
    global _CACHED_NC
    if _CACHED_NC is None:
        _CACHED_NC = build_nc()
    return _CACHED_NC


def make_in_maps(hidden_states, Wq, Wk, Wv, Wo, b_out):
    hs = np.asarray(hidden_states, dtype=np.float32)
    bf = ml_dtypes.bfloat16
    wqT = np.ascontiguousarray(np.asarray(Wq, np.float32).T).astype(bf)
    wkT = np.ascontiguousarray(np.asarray(Wk, np.float32).T).astype(bf)
    wvT = np.ascontiguousarray(np.asarray(Wv, np.float32).T).astype(bf)
    woT = np.ascontiguousarray(np.asarray(Wo, np.float32).T).astype(bf)
    bias = np.asarray(b_out, np.float32).reshape(C, 1)
    in_maps = []
    for c in range(NCORES):
        b, g = divmod(c, GROUP)
        i0 = g * SQ
        hsTb = hs[b].T  # [C, S]
        in_maps.append(
            {
                "hsT": np.ascontiguousarray(np.roll(hsTb, -i0, axis=1)).astype(bf),
                "res": np.ascontiguousarray(hsTb[:, i0 : i0 + SQ]) + bias,
                "wqT": wqT,
                "wkT": wkT,
                "wvT": wvT,
                "woT": woT,
            }
        )
    return in_maps


def assemble(results) -> np.ndarray:
    y = np.empty((B, S, C), np.float32)
    for c in range(NCORES):
        b, g = divmod(c, GROUP)
        i0 = g * SQ
        y[b, i0 : i0 + SQ, :] = np.asarray(results[c]["out"], np.float32).T
    return y


def kernel(**inputs) -> np.ndarray:
    from concourse.bass_utils import run_bass_kernel_spmd

    nc = get_nc()
    in_maps = make_in_maps(**inputs)
    res = run_bass_kernel_spmd(nc, in_maps, list(range(NCORES)))
    return assemble(res.results)


if __name__ == "__main__":
    import reference

    inputs = {k: np.asarray(v) for k, v in reference.setup_inputs().items()}
    got = kernel(**inputs)
    want = np.asarray(reference.reference(**inputs))
    err = np.linalg.norm(got - want) / np.linalg.norm(want)
    print("Relative error:", err)


# revision 31
# speedup vs baseline: 1.0818x; 1.0076x over previous
"""Multi-head attention (AttnProcessor2_0) on 8 TRN2 NeuronCores.

Problem: B=2, S=4096, C=640, H=10, Dh=64.
  q/k/v = hs @ W{q,k,v}.T ; per-head scores = q k^T / 8 ; softmax ;
  out = probs v ; y = out @ Wo.T + b_out + hs

Sharding (no collectives): core c -> batch b=c//4, query block g=c%4
(1024 queries).  Each core recomputes full K/V for its batch (head-dim
on partitions), computes its own S/4 x S attention block, output
projection, bias+residual.  Host passes hidden states TRANSPOSED and
ROLLED by the query offset so the same SPMD program works on every
core (softmax+PV are permutation-invariant along the key axis).

Device layout (everything feature-on-partition, token-on-free):
  kT [640, 4096] (5 chunks of 128 = 2 heads each)  "scoresT" = K Q^T
  qT [128, 1024] per head pair (rows 0:64 head even, 64:128 head odd)
  v  [4096, 650] (65-stride per head: 64 cols + ones col -> softmax
     denominators fall out of the PV matmul as PSUM row 64)
  QK: both heads of a pair run CONCURRENTLY as K=64 row-tiled matmuls
     (tile_position (0,0) and (64,0)) writing adjacent PSUM banks --
     2x the padded-contraction QK throughput of the old layout.
  probs: scoresT in PSUM -> ScalarE exp -> bf16 SBUF
  normalization: pv [65,512] copied to SBUF right after the PV
     accumulation stops (frees the PSUM bank), reciprocal of denom row,
     rank-1 PE outer product into a scratch PSUM bank to broadcast
     across partitions, DVE mult.
All matmuls bf16 (f32 PSUM accumulation).
"""

import sys

if "/opt/trn_rl_repo" not in sys.path:
    sys.path.insert(0, "/opt/trn_rl_repo")

from collections import deque
from contextlib import ExitStack

import ml_dtypes
import numpy as np

import concourse.bass as bass
import concourse.tile as tile
from concourse import mybir
from concourse.bass import ts

BF16 = mybir.dt.bfloat16
F32 = mybir.dt.float32
F8 = mybir.dt.float8e4

B, S, C = 2, 4096, 640
H, DH = 10, 64
NCORES = 8
GROUP = 4  # cores per batch element
SQ = S // GROUP  # 1024 queries per core
SCALE = 0.125  # 1/sqrt(64)
CCH = C // 128  # 5 feature chunks (2 heads each)
NJT = S // 512  # 8 key tiles for K proj
NJC = S // 128  # 32 key chunks for attention
NIT = SQ // 512  # 2 query tiles
VST = DH + 1  # 65: per-head stride in v tiles (ones col appended)

# hs/Wq/Wk/Wv are fed to the projections in fp8e4 (DoubleRow packs two
# 128-feature contraction chunks per matmul -> ~1.8x projection speed).
# The weights are scaled x32 on the host so they sit in fp8's normal
# range (std 0.02 -> 0.64; below 2^-6 e4m3 goes subnormal); the factor
# cancels exactly: q,k 32x -> scores 1024x (folded into the exp scale,
# a power of two), v 32x -> numerator and... the ones-column denominator
# is unscaled, so attn comes out 32x and Wo is pre-divided by 32.
WSCALE = 32.0
SCALE_EXP = SCALE / (WSCALE * WSCALE)  # exp scale on raw fp8 scores

# Schraudolph exp offload: selected score chunks compute exp on the DVE
# as a bf16 bit-trick (one tensor_scalar: bits = round(s*A + B) viewed as
# bf16 gives 2^(s*log2e) with ~2% per-element jitter and ~zero mean; any
# constant bias cancels in the softmax ratio).  This moves work off the
# bottleneck ScalarE onto the (slack) DVE.  Set empty to disable.
SCHRAUD_A = SCALE_EXP * 128.0 / float(np.log(2.0))
SCHRAUD_B = 127.0 * 128.0 - 7.45
OFFLOAD_JC = frozenset((2, 6, 10, 14, 18, 22, 26, 30))


def build_nc() -> bass.Bass:
    nc = bass.Bass()
    hsT = nc.declare_dram_parameter("hsT", [C, S], F8, isOutput=False)
    res = nc.declare_dram_parameter("res", [C, SQ], F32, isOutput=False)
    wqT = nc.declare_dram_parameter("wqT", [C, C], F8, isOutput=False)
    wkT = nc.declare_dram_parameter("wkT", [C, C], F8, isOutput=False)
    wvT = nc.declare_dram_parameter("wvT", [C, C], F8, isOutput=False)
    woT = nc.declare_dram_parameter("woT", [C, C], BF16, isOutput=False)
    out = nc.declare_dram_parameter("out", [C, SQ], F32, isOutput=True)

    with ExitStack() as ctx:
        tc = ctx.enter_context(tile.TileContext(nc))
        # outer pool: tensors whose lifetime spans projections AND attention
        sb = ctx.enter_context(tc.tile_pool(name="sb", bufs=1))

        kT_sb = [sb.tile([128, S], BF16, tag=f"kT{i}", name=f"kT{i}") for i in range(CCH)]
        # head-pair q: rows 0:64 = even head, 64:128 = odd head.  The QK
        # matmuls are K=64 row-tiled (tile_position (0,0)/(64,0)) and run
        # concurrently in the PE array -- no zero padding needed.
        qT_sb = [sb.tile([128, SQ], BF16, tag=f"qT{i}", name=f"qT{i}") for i in range(CCH)]
        v_sb = [sb.tile([128, H * VST], BF16, tag=f"v{j}", name=f"v{j}") for j in range(NJC)]
        ones_sb = sb.tile([128, DH], BF16, tag="ones", name="ones")
        nc.vector.memset(ones_sb[:], 1.0)

        # prefetch the exp table set while DMAs stream (the pseudo
        # ACT_TABLE_LOAD walrus inserts before the first real exp would
        # otherwise land on the critical path, ~1.3us)
        warm = sb.tile([1, 16], F32, tag="warm", name="warm")
        nc.vector.memset(warm[:], 0.0)
        nc.scalar.activation(warm[:], warm[:],
                             mybir.ActivationFunctionType.Exp,
                             bias=0.0, scale=0.0)

        # ---------------- load + first projections ----------------
        # Each input tensor is ONE wide SBUF tile filled by ONE DMA (the
        # Sync engine issues triggers at ~600ns each -- 20 small DMAs
        # serialized the old startup).  Chunk cc of a tensor lives at
        # free-offset cc*width; h3/wk3/... are [128, chunk, width] views.
        # fp8 operand tiles are padded to SIX feature chunks (chunk 5
        # zeroed) so every projection matmul group is uniformly DoubleRow
        # -- mixing DoubleRow and plain matmuls in one accumulation group
        # hard-hangs the PE (NRT_EXEC_UNIT_UNRECOVERABLE).
        load = ctx.enter_context(tc.tile_pool(name="load", bufs=1))
        CC6 = CCH + 1
        hsT_big = load.tile([128, CC6 * S], F8, tag="hsT", name="hsT")
        h3 = hsT_big[:].rearrange("p (f s) -> p f s", s=S)
        wk3 = load.tile([128, CC6 * C], F8, tag="wk", name="wk")[:] \
            .rearrange("p (f c) -> p f c", c=C)
        wq3 = load.tile([128, CC6 * C], F8, tag="wq", name="wq")[:] \
            .rearrange("p (f c) -> p f c", c=C)
        wv3 = load.tile([128, CC6 * C], F8, tag="wv", name="wv")[:] \
            .rearrange("p (f c) -> p f c", c=C)
        nc.vector.memset(h3[:, CCH, :], 0.0)
        nc.vector.memset(wk3[:, CCH, :], 0.0)
        nc.vector.memset(wq3[:, CCH, :], 0.0)
        nc.vector.memset(wv3[:, CCH, :], 0.0)
        # full Wo resident (800KB): kills the per-oproj weight DMAs and
        # zero-padding; with head-paired attn the contraction is all-real
        wo3 = load.tile([128, CCH * C], BF16, tag="wo", name="wo")[:] \
            .rearrange("p (f c) -> p f c", c=C)
        nc.sync.dma_start(wk3[:, 0:CCH, :], wkT[:, :].rearrange("(f p) c -> p f c", p=128))
        nc.sync.dma_start(
            h3[:, 0:CCH, 0:SQ],
            hsT[:, 0:SQ].rearrange("(f p) s -> p f s", p=128),
        )
        nc.sync.dma_start(wq3[:, 0:CCH, :], wqT[:, :].rearrange("(f p) c -> p f c", p=128))
        nc.sync.dma_start(wv3[:, 0:CCH, :], wvT[:, :].rearrange("(f p) c -> p f c", p=128))
        nc.sync.dma_start(wo3, woT[:, :].rearrange("(f p) c -> p f c", p=128))

        def emit_hsT_tail():
            # deferred until after the first exp so ScalarE's conservative
            # vector-clock waits don't cover this 4MB of DMA
            for blk in range(SQ, S, SQ):
                nc.sync.dma_start(
                    h3[:, 0:CCH, blk : blk + SQ],
                    hsT[:, blk : blk + SQ].rearrange("(f p) s -> p f s", p=128),
                )

        def proj_ops(w3, dst, dc, jt, pool):
            # one K/Q projection stripe as micro-ops (2 DoubleRow fp8 MMs
            # covering feature chunks 0-3 + 1 plain MM for chunk 4 +
            # cast), so the background drain never inserts a long matmul
            # burst between attention-stream matmuls
            st = {}

            def need_ps():
                if "ps" not in st:
                    st["ps"] = pool.tile([128, 512], F32, tag="pp",
                                         name="pp", bufs=2)

            def mm2(cc):
                def f():
                    need_ps()
                    nc.tensor.matmul(
                        st["ps"][:],
                        w3[:, cc : cc + 2, ts(dc, 128)],
                        h3[:, cc : cc + 2, ts(jt, 512)],
                        start=(cc == 0),
                        stop=(cc == 4),
                        perf_mode=mybir.MatmulPerfMode.DoubleRow,
                    )
                return f

            def cast():
                nc.vector.tensor_copy(dst[:, ts(jt, 512)], st["ps"][:])

            return [mm2(0), mm2(2), mm2(4), cast]

        def emit_kproj(dc, jt, pool):
            for f in proj_ops(wk3, kT_sb[dc], dc, jt, pool):
                f()

        def emit_qproj(dc, it, pool):
            for f in proj_ops(wq3, qT_sb[dc], dc, it, pool):
                f()

        def emit_vproj(jc, pool):
            vt = v_sb[jc]
            v3 = vt[:].rearrange("p (h x) -> p h x", x=VST)
            for d0, dn in ((0, 512), (512, 128)):
                ps = pool.tile([128, 512], F32, tag="pp", name="pp", bufs=2)
                for cc in (0, 2, 4):
                    nc.tensor.matmul(
                        ps[:, 0:dn],
                        h3[:, cc : cc + 2, ts(jc, 128)],
                        wv3[:, cc : cc + 2, d0 : d0 + dn],
                        start=(cc == 0),
                        stop=(cc == 4),
                        perf_mode=mybir.MatmulPerfMode.DoubleRow,
                    )
                nc.vector.tensor_copy(
                    v3[:, d0 // DH : (d0 + dn) // DH, 0:DH],
                    ps[:, 0:dn].rearrange("p (h x) -> p h x", x=DH),
                )

        # ones columns of all v tiles set once up front (DVE is idle
        # during the DMA-bound startup; doing this inside window 0 cost
        # ~0.7us of DVE per chunk right where the PE is most oversubscribed)
        for jc in range(NJC):
            v3c = v_sb[jc][:].rearrange("p (h x) -> p h x", x=VST)
            nc.vector.memset(v3c[:, :, DH : DH + 1], 1.0)

        with tc.tile_pool(name="pp0", bufs=2, space="PSUM") as pp0:
            for jt in range(2):
                emit_kproj(0, jt, pp0)
            emit_qproj(0, 0, pp0)

        # ---------------- attention phase ----------------
        # attn2[hp]: head pair packed (rows 0:64 even head, 64:128 odd) --
        # the output projection contracts all 128 rows with no padding
        attn2_sb = [sb.tile([128, SQ], BF16, tag=f"attn{p}", name=f"attn{p}")
                    for p in range(CCH)]
        with tc.tile_pool(name="ap", bufs=1, space="PSUM") as ap, \
             tc.tile_pool(name="pt", bufs=6) as pt_pool, \
             tc.tile_pool(name="ob", bufs=3) as ob, \
             tc.tile_pool(name="scratch", bufs=3) as scratch:
            def norm_dve(hp, pv0, pv1, p_isl, tail=False):
                # drain both pv accumulators into one packed tile (DVE
                # copies may shift partitions), denominators to rows 0/32
                # of a shared tile -> ONE reciprocal per head pair
                rawp = scratch.tile([128, 512], BF16, tag="raw", name="raw",
                                    bufs=2)
                nc.vector.tensor_copy(rawp[0:DH, :], pv0[0:DH, :])
                nc.vector.tensor_copy(rawp[DH:128, :], pv1[0:DH, :])
                rc = scratch.tile([33, 512], BF16, tag="rc", name="rc",
                                  bufs=2)
                dn = scratch.tile([33, 512], BF16, tag="dn", name="dn",
                                  bufs=2)
                nc.vector.tensor_copy(dn[0:1, :], pv0[DH : DH + 1, :])
                nc.vector.tensor_copy(dn[32:33, :], pv1[DH : DH + 1, :])
                with nc.allow_low_precision(reason="softmax recip bf16"):
                    if tail:
                        # tail: ScalarE is idle and the DVE reciprocal
                        # (3.3us, 8 cyc/elem iterative divide) would gate
                        # the final output projection; 1/x = exp(-ln(x))
                        # costs 2 short ACTs instead
                        lg = scratch.tile([33, 512], F32, tag="lg",
                                          name="lg", bufs=2)
                        nc.scalar.activation(
                            lg[:], dn[:], mybir.ActivationFunctionType.Ln,
                            bias=0.0, scale=1.0,
                        )
                        nc.scalar.activation(
                            rc[:], lg[:], mybir.ActivationFunctionType.Exp,
                            bias=0.0, scale=-1.0,
                        )
                    else:
                        nc.vector.reciprocal(rc[:], dn[:])
                return (hp, p_isl, rc, rawp)

            def norm_pe(state, anchor, pool):
                # two concurrent rank-1 PE broadcasts of the reciprocals
                # (row/col tiles (0,0) and (32,64)), pinned behind the
                # anchor QK so the slow DVE reciprocal is hidden
                hp, p_isl, rc, rawp = state
                ps = pool.tile([128, 512], F32, tag="pp", name="pp", bufs=2)
                r_mm = nc.tensor.matmul(
                    ps[0:DH, :],
                    ones_sb[0:1, 0:DH],
                    rc[0:1, :],
                    start=True,
                    stop=True,
                )
                nc.tensor.matmul(
                    ps[DH:128, :],
                    ones_sb[32:33, 0:DH],
                    rc[32:33, :],
                    start=True,
                    stop=True,
                )
                if anchor is not None:
                    tile.add_dep_helper(
                        r_mm.ins, anchor.ins, sync=False,
                        reason="norm R after anchor QK (hide recip latency)",
                    )
                nc.vector.tensor_mul(
                    attn2_sb[hp][:, p_isl], rawp[:], ps[:]
                )

            def oproj_ops(ec, it):
                st = {}

                def mm(r):
                    def f():
                        if "ps" not in st:
                            st["ps"] = ap.tile([128, 512], F32, tag="pp",
                                               name="pp", bufs=2)
                            rt = ob.tile([128, 512], F32, tag="rt",
                                         name="rt", bufs=2)
                            nc.sync.dma_start(
                                rt[:], res[ts(ec, 128), ts(it, 512)]
                            )
                            st["rt"] = rt
                        nc.tensor.matmul(
                            st["ps"][:],
                            wo3[:, r, ts(ec, 128)],
                            attn2_sb[r][:, ts(it, 512)],
                            start=(r == 0),
                            stop=(r == CCH - 1),
                        )
                    return f

                def fin():
                    ot = ob.tile([128, 512], F32, tag="ot", name="ot", bufs=2)
                    nc.vector.tensor_add(ot[:], st["ps"][:], st["rt"][:])
                    nc.sync.dma_start(out[ts(ec, 128), ts(it, 512)], ot[:])

                return [mm(r) for r in range(CCH)] + [fin]

            def emit_oproj(ec, it):
                for f in oproj_ops(ec, it):
                    f()

            # Background work flows through a micro-op queue drained at
            # most 2 ops per jc step, so no more than ~2 weight matmuls
            # ever sit between attention-stream matmuls on the PE (a
            # whole 5-MM projection burst stalls the exp pipeline).
            # Deadlines: a window's own kT stripes jt2..7 (first used at
            # jc8/12/../28) are enqueued at window start and finish by
            # ~jc17; the next window's kT jt0/jt1 + qT slice drain by the
            # window's end.  it1 windows carry the it0 output projections
            # and lazily-deferred it1 q-projections.
            bgq = deque()

            pending = []
            for it in range(NIT):
                isl = ts(it, 512)
                for hp in range(CCH):
                    h0, h1 = 2 * hp, 2 * hp + 1
                    if it == 0:
                        for jt in range(2, NJT):
                            bgq.extend(proj_ops(wk3, kT_sb[hp], hp, jt, ap))
                        if hp < CCH - 1:
                            for jt in range(2):
                                bgq.extend(
                                    proj_ops(wk3, kT_sb[hp + 1], hp + 1, jt, ap)
                                )
                            bgq.extend(proj_ops(wq3, qT_sb[hp + 1], hp + 1, 0, ap))
                        else:
                            bgq.extend(proj_ops(wq3, qT_sb[0], 0, 1, ap))
                    else:
                        if hp == 0:
                            bgq.extend(proj_ops(wq3, qT_sb[1], 1, 1, ap))
                        elif hp == 1:
                            bgq.extend(oproj_ops(0, 0))
                            bgq.extend(oproj_ops(1, 0))
                            bgq.extend(proj_ops(wq3, qT_sb[2], 2, 1, ap))
                        elif hp == 2:
                            bgq.extend(oproj_ops(2, 0))
                            bgq.extend(proj_ops(wq3, qT_sb[3], 3, 1, ap))
                        elif hp == 3:
                            bgq.extend(oproj_ops(3, 0))
                            bgq.extend(proj_ops(wq3, qT_sb[4], 4, 1, ap))
                        else:
                            bgq.extend(oproj_ops(4, 0))
                    vtodo = {}
                    if it == 0 and hp == 0:
                        # V chunks 0..7 front-loaded (keys 0:1024 resident
                        # before the hsT tail lands), then one chunk per
                        # step four steps ahead of its PV use
                        for jc in range(4):
                            vtodo[jc] = [2 * jc, 2 * jc + 1]
                        for jc in range(4, 28):
                            vtodo[jc] = [jc + 4]
                    pv0 = ap.tile([DH + 1, 512], F32, tag="pv0", bufs=1,
                                  name="pv0")
                    pv1 = ap.tile([DH + 1, 512], F32, tag="pv1", bufs=1,
                                  name="pv1")
                    for jc in range(NJC):
                        sc = ap.tile([128, 1024], F32, tag="sc", bufs=2,
                                     name="sc")
                        qk0 = nc.tensor.matmul(
                            sc[:, 0:512],
                            kT_sb[hp][0:DH, ts(jc, 128)],
                            qT_sb[hp][0:DH, isl],
                            start=True,
                            stop=True,
                        )
                        nc.tensor.matmul(
                            sc[:, 512:1024],
                            kT_sb[hp][DH:128, ts(jc, 128)],
                            qT_sb[hp][DH:128, isl],
                            start=True,
                            stop=True,
                        )
                        pt = pt_pool.tile([128, 1024], BF16,
                                          tag="pt", name="pt")
                        if (it, hp) != (0, 0) and jc in OFFLOAD_JC:
                            nc.vector.tensor_scalar(
                                out=pt[:].bitcast(mybir.dt.int16),
                                in0=sc[:],
                                scalar1=SCHRAUD_A,
                                scalar2=SCHRAUD_B,
                                op0=mybir.AluOpType.mult,
                                op1=mybir.AluOpType.add,
                            )
                        else:
                            nc.scalar.activation(
                                pt[:], sc[:],
                                mybir.ActivationFunctionType.Exp,
                                bias=0.0, scale=SCALE_EXP,
                            )
                        if it == 0 and hp == 0 and jc == 0:
                            emit_hsT_tail()
                        if pending and jc == 5:
                            norm_pe(pending.pop(0), qk0, ap)
                        for j in vtodo.get(jc, ()):
                            emit_vproj(j, ap)
                        for _ in range(2):
                            if bgq:
                                bgq.popleft()()
                        nc.tensor.matmul(
                            pv0[:],
                            v_sb[jc][:, h0 * VST : (h0 + 1) * VST],
                            pt[:, 0:512],
                            start=(jc == 0),
                            stop=(jc == NJC - 1),
                        )
                        nc.tensor.matmul(
                            pv1[:],
                            v_sb[jc][:, h1 * VST : (h1 + 1) * VST],
                            pt[:, 512:1024],
                            start=(jc == 0),
                            stop=(jc == NJC - 1),
                        )
                    pending.append(norm_dve(hp, pv0, pv1, isl,
                                            tail=(it == 1 and hp == CCH - 1)))
            while bgq:
                bgq.popleft()()
            # tail: the final pair's attn2[4] gates only the r=4 matmul of
            # each output projection -- accumulate r=0..3 for one ec (one
            # pp PSUM slot; the other must stay free for the norm's R)
            # underneath the reciprocal, then finish
            tail_ops = [oproj_ops(ec, 1) for ec in range(CCH)]
            for f in tail_ops[0][0:4]:
                f()
            for st in pending:
                norm_pe(st, None, ap)
            for ec in range(CCH):
                for f in tail_ops[ec][4:] if ec < 1 else tail_ops[ec]:
                    f()

    _spill_matmul_waits(nc)
    return nc


# walrus embedded-sync-wait capacity per BIR opcode.  Matmult holds a
# single wait; excess waits hoist onto the paired Ldweights (in-order
# issue on PE makes that equivalent).  Other compute ops spill onto
# EventSemaphore carrier instructions inserted just before them on the
# same engine.  DMACopy / Drain / EventSemaphore handle many waits
# natively (bacc emits such itself) and are left alone.
_WAIT_CAPS = {
    "InstMatmult": 1,
    "InstLdweights": 1,
    "InstActivation": 1,
    "InstReciprocal": 1,
    "InstTensorTensor": 1,
    "InstTensorCopy": 1,
    "InstTensorScalarPtr": 1,
    "InstTensorReduce": 1,
    "InstMemset": 1,
    "InstDMACopy": 1,
    "InstDrain": 1,
    "InstCustomDveAnt": 1,
}
_ES_CAP = 2  # waits per EventSemaphore carrier (walrus: <=2 waits, <=1 update)


def _spill_matmul_waits(nc: bass.Bass) -> None:
    spill_id = [0]

    def carriers(excess, engine):
        out = []
        for i in range(0, len(excess), _ES_CAP):
            es = mybir.InstEventSemaphore(
                name=f"wait-spill-{spill_id[0]}", ins=[], outs=[]
            )
            spill_id[0] += 1
            es.engine = engine
            es.sync_info = mybir.SyncInfo(
                on_wait=excess[i : i + _ES_CAP], on_update=[]
            )
            out.append(es)
        return out

    for f in nc.m.functions:
        for blk in f.blocks:
            insts = blk.instructions
            i = 0
            while i < len(insts):
                inst = insts[i]
                tn = type(inst).__name__
                cap = _WAIT_CAPS.get(tn)
                si = inst.sync_info
                if cap is None or si is None or len(si.on_wait) <= cap:
                    i += 1
                    continue
                w = list(si.on_wait)
                if tn == "InstMatmult" and cap == 1:
                    # Keep the latest-satisfied dependency (the ACT-produced
                    # operand, e.g. probs from exp) embedded on the matmul and
                    # hoist early ones onto the Ldweights: a wait on the LDW
                    # blocks its background prefetch and serializes ~50ns of
                    # weight-load into every PV matmul.
                    acts = [x for x in w if "Activation" in (x.ant_name or "")]
                    if acts:
                        keep = [acts[-1]]
                        excess = [x for x in w if x is not acts[-1]]
                    else:
                        keep, excess = w[-cap:], w[:-cap]
                else:
                    keep, excess = w[-cap:], w[:-cap]
                prev = insts[i - 1] if i > 0 else None
                if (
                    tn == "InstMatmult"
                    and prev is not None
                    and type(prev).__name__ == "InstLdweights"
                    and len(((prev.sync_info and prev.sync_info.on_wait) or []))
                    + len(excess) <= 1
                ):
                    psi = prev.sync_info
                    pw = list(psi.on_wait) if psi is not None else []
                    pu = list(psi.on_update) if psi is not None else []
                    prev.sync_info = mybir.SyncInfo(on_wait=pw + excess, on_update=pu)
                else:
                    new = carriers(excess, inst.engine)
                    insts[i:i] = new
                    i += len(new)
                inst.sync_info = mybir.SyncInfo(
                    on_wait=keep, on_update=list(si.on_update)
                )
                i += 1


_CACHED_NC = None


def get_nc() -> bass.Bass:
    global _CACHED_NC
    if _CACHED_NC is None:
        _CACHED_NC = build_nc()
    return _CACHED_NC


def make_in_maps(hidden_states, Wq, Wk, Wv, Wo, b_out):
    hs = np.asarray(hidden_states, dtype=np.float32)
    bf = ml_dtypes.bfloat16
    f8 = ml_dtypes.float8_e4m3
    # projection weights x32 into fp8's normal range (cancelled by the
    # exp scale / the ones-column denominator / Wo pre-divided by 32)
    wqT = np.ascontiguousarray(np.asarray(Wq, np.float32).T * WSCALE).astype(f8)
    wkT = np.ascontiguousarray(np.asarray(Wk, np.float32).T * WSCALE).astype(f8)
    wvT = np.ascontiguousarray(np.asarray(Wv, np.float32).T * WSCALE).astype(f8)
    woT = np.ascontiguousarray(np.asarray(Wo, np.float32).T / WSCALE).astype(bf)
    bias = np.asarray(b_out, np.float32).reshape(C, 1)
    in_maps = []
    for c in range(NCORES):
        b, g = divmod(c, GROUP)
        i0 = g * SQ
        hsTb = hs[b].T  # [C, S]
        in_maps.append(
            {
                "hsT": np.ascontiguousarray(np.roll(hsTb, -i0, axis=1)).astype(f8),
                "res": np.ascontiguousarray(hsTb[:, i0 : i0 + SQ]) + bias,
                "wqT": wqT,
                "wkT": wkT,
                "wvT": wvT,
                "woT": woT,
            }
        )
    return in_maps


def assemble(results) -> np.ndarray:
    y = np.empty((B, S, C), np.float32)
    for c in range(NCORES):
        b, g = divmod(c, GROUP)
        i0 = g * SQ
        y[b, i0 : i0 + SQ, :] = np.asarray(results[c]["out"], np.float32).T
    return y


def kernel(**inputs) -> np.ndarray:
    from concourse.bass_utils import run_bass_kernel_spmd

    nc = get_nc()
    in_maps = make_in_maps(**inputs)
    res = run_bass_kernel_spmd(nc, in_maps, list(range(NCORES)))
    return assemble(res.results)


if __name__ == "__main__":
    import reference

    inputs = {k: np.asarray(v) for k, v in reference.setup_inputs().items()}
    got = kernel(**inputs)
    want = np.asarray(reference.reference(**inputs))
    err = np.linalg.norm(got - want) / np.linalg.norm(want)
    print("Relative error:", err)
